# revision 1
# baseline (speedup 1.0000x reference)
"""Trainium2 Bass kernel for nn_GCNWithMultiHeadGATAndTCN_42356967473538.

Sharding: 8 cores = (batch b in 0..3) x (node-half s in 0..1).
Each core computes its 1024 node rows of its batch through the whole
pipeline, channels-major ([channel partitions, node free]) so BatchNorm
scales are per-partition and the TCN conv contracts on partitions.

Cross-core communication (training-mode BatchNorm couples all batches):
  C1: AllReduce [128,4]   bn1 sums           (all 8 cores)
  C2: AllGather [128,2048] h_bn^T            (pairs: other node half)
  C3: AllGather [128,4]   g boundary columns (pairs: conv halo)
  C4: AllReduce [128,4]   bn2 sums           (all 8 cores)
"""

import numpy as np

import concourse.bass as bass
import concourse.mybir as mybir
import concourse.tile as tile
from concourse import bacc, dve_ops
from concourse.bass_utils import run_bass_kernel_spmd
from concourse.dve_spec import Spec, Src0, C0, maxx, lower, _has_src1
from concourse.dve_uop import DveOpSpec
from concourse.dve_table_gen import dve_ver_for


def _register_lrelu_op():
    """Custom single-pass DVE leaky-relu: out = max(in0, in0*s0)."""
    if "LRELU_ANT" in dve_ops._SUB_OPCODE_FOR_NAME:
        return dve_ops.CUSTOM_DVE_SPECS and next(
            op for op in dve_ops.OPS if op.name == "LRELU_ANT"
        )
    spec = Spec(
        body=maxx(Src0, Src0 * C0),
        reference=lambda in0, in1, s0, s1, imm2: np.maximum(
            np.nan_to_num(in0, nan=0.0, posinf=np.inf, neginf=-np.inf),
            in0 * s0,
        ).astype(np.float32),
    )
    row = dve_ops._CUSTOM_DVE_ROW_BASE + len(dve_ops.OPS)
    assert row < 0x20
    shas = {}
    for ver in ("v3", "v4"):
        try:
            tmp = DveOpSpec(name="LRELU_ANT", opcode=row, uops=lower(spec, ver=ver),
                            rd1_en=_has_src1(spec))
            shas[ver] = tmp.sha(ver)
        except Exception:
            pass
    op = dve_ops.DveOp("LRELU_ANT", spec, False, shas)
    dve_ops.OPS.append(op)
    dve_ops.CUSTOM_DVE_SPECS["LRELU_ANT"] = spec
    dve_ops._SUB_OPCODE_FOR_NAME["LRELU_ANT"] = row
    return op


LRELU_ANT = _register_lrelu_op()

F32 = mybir.dt.float32
F32R = mybir.dt.float32r
AF = mybir.ActivationFunctionType
ALU = mybir.AluOpType
AX = mybir.AxisListType

B, N, FEAT, C, H, DH = 4, 2048, 256, 256, 4, 64
P = 128
R = N // 2            # own rows per core (1024)
NC = 8                # cores
EPS = 1e-5
SLOPE = 0.2
EXP_SHIFT = 64.0  # softmax-invariant constant shift: keeps exp in f32 range
CNT = float(B * N)    # batchnorm sample count (8192)

PAIRS = [[0, 1], [2, 3], [4, 5], [6, 7]]
ALL8 = [list(range(NC))]


def _bc_ap(ap, parts=P):
    """Broadcast a DRAM AP across `parts` partitions (stride-0 partition dim)."""
    return bass.AP(tensor=ap.tensor, offset=ap.offset, ap=[[0, parts], *ap.ap])


def build_program(alpha_gat: float, alpha_tcn: float, sim_safe: bool = False,
                  debug_taps: bool = False):
    nc = bacc.Bacc(
        "TRN2", target_bir_lowering=False, debug=False, num_devices=NC
    )

    def din(name, shape, dt=F32):
        return nc.dram_tensor(name, shape, dt, kind="ExternalInput").ap()

    xT = din("xT", [FEAT, N], F32R)      # x[b].T
    adjTc = din("adjTc", [N, R], F32R)   # adj[s*R:(s+1)*R, :].T  (own columns)
    W = din("W", [FEAT, C], F32R)        # W_sage
    bs = din("bs", [C])
    g1 = din("g1", [C])
    b1 = din("b1", [C])
    Whp = din("Whp", [C, H * DH], F32R)        # Wh packed [j, h*64+d]
    WkT = din("WkT", [3, C, C], F32R)          # conv_w[:, :, k].T -> [k, cin, cout]
    g2 = din("g2", [C])
    b2 = din("b2", [C])
    Ebc = din("Ebc", [4, C], F32R)             # head->channel one-hot (recip bcast)
    hmask = din("hmask", [2, 2, 2])      # halo select [L/R, src, first/last]

    out = nc.dram_tensor("out", [P, 2, R], F32, kind="ExternalOutput").ap()
    taps = {}
    if debug_taps:
        for tn, shape in [("d_hT", [P, 2, R]), ("d_hbnT", [P, 2, R]),
                          ("d_hpTo", [P, 2, R]), ("d_aggT", [P, 2, R]),
                          ("d_den4", [4, 2 * 512]), ("d_g", [P, 2, R + 2]),
                          ("d_tconv", [P, 2, R]), ("d_rec4", [4, 2 * 512]),
                          ("d_st1", [P, 4]), ("d_stg1", [P, 4])]:
            taps[tn] = nc.dram_tensor(tn, shape, F32, kind="ExternalOutput").ap()

    # internal DRAM bounce buffers for collectives
    def dbuf(name, shape):
        return nc.dram_tensor(name, shape, F32).ap()

    cc1_in = dbuf("cc1_in", [P, 4])
    cc1_out = dbuf("cc1_out", [P, 4])
    cc2_in = nc.dram_tensor("cc2_in", [P, 2 * R], F32R).ap()
    cc2_out = nc.dram_tensor("cc2_out", [2, P, 2 * R], F32R).ap()
    cc3_in = dbuf("cc3_in", [P, 4])
    cc3_out = dbuf("cc3_out", [2, P, 4])
    cc4_in = dbuf("cc4_in", [P, 4])
    cc4_out = dbuf("cc4_out", [P, 4])


    with tile.TileContext(nc) as tc:
        with (
            tc.tile_pool(name="persist", bufs=1) as ppool,
            tc.tile_pool(name="work", bufs=2) as wpool,
            tc.tile_pool(name="adjp", bufs=3) as adjpool,
            tc.tile_pool(name="expp", bufs=3) as expool,
            tc.tile_pool(name="psum", bufs=1, space="PSUM") as psum,
        ):
            # ---------- constants ----------
            W_sb = ppool.tile([P, 2, C], F32R, tag="W_sb")
            nc.sync.dma_start(W_sb[:], W.rearrange("(o p) c -> p o c", p=P))
            Wh_sb = ppool.tile([P, 2, C], F32R, tag="Wh_sb")
            nc.sync.dma_start(Wh_sb[:], Whp.rearrange("(o p) c -> p o c", p=P))
            Wk_sb = ppool.tile([P, 2, 3, C], F32R, tag="Wk_sb")
            for k in range(3):
                nc.sync.dma_start(
                    Wk_sb[:, :, k, :],
                    WkT[k].rearrange("(o p) c -> p o c", p=P),
                )
            bs_sb = ppool.tile([P, 2], F32, tag="bs_sb")
            nc.sync.dma_start(bs_sb[:], bs.rearrange("(o p) -> p o", p=P))
            g1_sb = ppool.tile([P, 2], F32, tag="g1_sb")
            nc.sync.dma_start(g1_sb[:], g1.rearrange("(o p) -> p o", p=P))
            b1_sb = ppool.tile([P, 2], F32, tag="b1_sb")
            nc.sync.dma_start(b1_sb[:], b1.rearrange("(o p) -> p o", p=P))
            g2_sb = ppool.tile([P, 2], F32, tag="g2_sb")
            nc.sync.dma_start(g2_sb[:], g2.rearrange("(o p) -> p o", p=P))
            b2_sb = ppool.tile([P, 2], F32, tag="b2_sb")
            nc.sync.dma_start(b2_sb[:], b2.rearrange("(o p) -> p o", p=P))
            Ebc_sb = ppool.tile([4, C], F32R, tag="Ebc_sb")
            nc.sync.dma_start(Ebc_sb[:], Ebc[:, :])
            hm_sb = ppool.tile([P, 2, 2, 2], F32, tag="hm_sb")
            nc.sync.dma_start(hm_sb[:], _bc_ap(hmask[:, :, :]))
            cm40 = ppool.tile([P, 1], F32, tag="cm40")
            nc.vector.memset(cm40[:], -EXP_SHIFT)

            # ---------- phase A: support = x @ W  (support[m, j], m on parts)
            support = ppool.tile([P, 16, C], F32R, tag="big16", name="support")
            xTv = xT.rearrange("(ko p) m -> p ko m", p=P)
            for t in range(16):
                ps = psum.tile([P, C], F32, tag=f"q{t % 2}", name="ps_sup")
                xt = wpool.tile([P, 2, P], F32R, tag="xt")
                nc.sync.dma_start(xt[:], xTv[:, :, t * P : (t + 1) * P])
                for ko in range(2):
                    nc.tensor.matmul(
                        ps[:], xt[:, ko, :], W_sb[:, ko, :],
                        start=(ko == 0), stop=(ko == 1),
                    )
                nc.vector.tensor_copy(out=support[:, t, :], in_=ps[:])

            # ---------- phase B: hT = relu(support^T @ adjT + b)  [j, n_own]
            hT = ppool.tile([P, 2, R], F32, tag="hT_share", name="hT")
            ps_h = [
                [
                    psum.tile([P, 512], F32, tag=f"q{o * 2 + w}", name=f"ps_h{o}{w}")
                    for w in range(2)
                ]
                for o in range(2)
            ]
            for t in range(16):
                at = adjpool.tile([P, R], F32R, tag="at")
                nc.sync.dma_start(at[:], adjTc[t * P : (t + 1) * P, :])
                for o in range(2):
                    for w in range(2):
                        nc.tensor.matmul(
                            ps_h[o][w][:],
                            support[:, t, o * P : (o + 1) * P],
                            at[:, w * 512 : (w + 1) * 512],
                            start=(t == 0), stop=(t == 15),
                        )
            for o in range(2):
                for w in range(2):
                    nc.scalar.activation(
                        out=hT[:, o, w * 512 : (w + 1) * 512],
                        in_=ps_h[o][w][:],
                        func=AF.Relu,
                        bias=bs_sb[:, o : o + 1],
                    )

            # ---------- phase C: BN1 stats + allreduce + apply
            st1 = ppool.tile([P, 4], F32, tag="st1")
            sq_scr = wpool.tile([P, R], F32, tag="sq_scr")
            for o in range(2):
                nc.vector.reduce_sum(st1[:, o : o + 1], hT[:, o, :], axis=AX.X)
                nc.scalar.activation(
                    out=sq_scr[:],
                    in_=hT[:, o, :],
                    func=AF.Square,
                    accum_out=st1[:, 2 + o : 3 + o],
                )
            nc.sync.dma_start(cc1_in[:, :], st1[:])
            nc.gpsimd.collective_compute(
                "AllReduce", ALU.add, replica_groups=ALL8,
                ins=[cc1_in.opt()], outs=[cc1_out.opt()],
            )
            stg1 = ppool.tile([P, 4], F32, tag="stg1")
            nc.sync.dma_start(stg1[:], cc1_out[:, :])

            def bn_affine(stg, gam, bet, tagp, fold=1.0):
                """A, C with y = relu(x*A + C) == relu(fold*bn(x))."""
                mean = ppool.tile([P, 2], F32, tag=f"{tagp}_mean")
                nc.vector.tensor_scalar_mul(mean[:], stg[:, 0:2], 1.0 / CNT)
                ex2 = ppool.tile([P, 2], F32, tag=f"{tagp}_ex2")
                nc.vector.tensor_scalar_mul(ex2[:], stg[:, 2:4], 1.0 / CNT)
                var = ppool.tile([P, 2], F32, tag=f"{tagp}_var")
                nc.vector.tensor_tensor(var[:], mean[:], mean[:], ALU.mult)
                nc.vector.tensor_tensor(var[:], ex2[:], var[:], ALU.subtract)
                rstd = ppool.tile([P, 2], F32, tag=f"{tagp}_rstd")
                nc.vector.tensor_scalar_add(var[:], var[:], EPS)
                nc.scalar.activation(rstd[:], var[:], AF.Ln)
                nc.scalar.activation(rstd[:], rstd[:], AF.Exp, scale=-0.5)
                A = ppool.tile([P, 2], F32, tag=f"{tagp}_A")
                nc.vector.tensor_tensor(A[:], gam[:], rstd[:], ALU.mult)
                Cc = ppool.tile([P, 2], F32, tag=f"{tagp}_C")
                nc.vector.tensor_tensor(Cc[:], mean[:], A[:], ALU.mult)
                nc.vector.tensor_tensor(Cc[:], bet[:], Cc[:], ALU.subtract)
                if fold != 1.0:
                    nc.vector.tensor_scalar_mul(A[:], A[:], fold)
                    nc.vector.tensor_scalar_mul(Cc[:], Cc[:], fold)
                return A, Cc

            A1, C1 = bn_affine(stg1, g1_sb, b1_sb, "bn1")
            hbnT = ppool.tile([P, 2, R], F32, tag="hbnT")
            for o in range(2):
                nc.scalar.activation(
                    out=hbnT[:, o, :], in_=hT[:, o, :], func=AF.Relu,
                    scale=A1[:, o : o + 1], bias=C1[:, o : o + 1],
                )

            # f32r copy of hbnT for matmul + gather use
            hbnT_r = ppool.tile([P, 2, R], F32R, tag="hbnT_r")
            nc.vector.tensor_copy(out=hbnT_r[:], in_=hbnT[:])

            # ---------- phase D: pair AllGather of hbnT
            nc.sync.dma_start(cc2_in.rearrange("p (o r) -> p o r", o=2), hbnT_r[:])
            nc.gpsimd.collective_compute(
                "AllGather", ALU.bypass, replica_groups=PAIRS,
                ins=[cc2_in.opt()], outs=[cc2_out.opt()],
            )
            hbnF = ppool.tile([P, 2, 2, R], F32R, tag="hbnF")  # [p, o, src, n]
            for src in range(2):
                nc.sync.dma_start(
                    hbnF[:, :, src, :],
                    cc2_out[src].rearrange("p (o r) -> p o r", o=2),
                )

            # ---------- phase E: hp projections
            # hpT_own: [d-major 128 (= head pair), nq local]
            hpTo = ppool.tile([P, 2, R], F32R, tag="hpTo")
            for hh in range(2):
                for w in range(2):
                    ps = psum.tile([P, 512], F32, tag=f"q{(hh * 2 + w) % 2}", name="ps_hpo")
                    for o in range(2):
                        nc.tensor.matmul(
                            ps[:],
                            Wh_sb[:, o, hh * P : (hh + 1) * P],
                            hbnT_r[:, o, w * 512 : (w + 1) * 512],
                            start=(o == 0), stop=(o == 1),
                        )
                    nc.vector.tensor_copy(
                        out=hpTo[:, hh, w * 512 : (w + 1) * 512], in_=ps[:]
                    )
            # hpT_full: [d-major, nk gathered 2048]
            hpTf = ppool.tile([P, 2, 2 * R], F32R, tag="big16", name="hpTf")
            for hh in range(2):
                for src in range(2):
                    for w in range(2):
                        ps = psum.tile([P, 512], F32, tag=f"q{(src * 2 + w) % 2}", name="ps_hpf")
                        for o in range(2):
                            nc.tensor.matmul(
                                ps[:],
                                Wh_sb[:, o, hh * P : (hh + 1) * P],
                                hbnF[:, o, src, w * 512 : (w + 1) * 512],
                                start=(o == 0), stop=(o == 1),
                            )
                        nc.vector.tensor_copy(
                            out=hpTf[
                                :, hh, src * R + w * 512 : src * R + (w + 1) * 512
                            ],
                            in_=ps[:],
                        )
            # hp node-major with ones columns: [nk, hh, 130]
            # cols 0:64 even head, 64 ones, 65:129 odd head, 129 ones
            # even lhsT = cols 0:65, odd lhsT = cols 65:130 -> both give
            # agg at psum partitions 0..63 and denominator at partition 64
            hpA = ppool.tile([P, 16, 2, 130], F32R, tag="hpA")
            ones1 = ppool.tile([P, 1], F32, tag="ones1")
            nc.vector.memset(ones1[:], 1.0)
            ones_src = bass.AP(
                tensor=ones1.tensor, offset=ones1.offset,
                ap=[ones1.ap[0], [0, 16], [0, 2]],
            )
            for col in (64, 129):
                onesv = bass.AP(
                    tensor=hpA.tensor, offset=hpA.offset + col,
                    ap=[hpA.ap[0], [260, 16], [130, 2]],
                )  # [p, t, hh] at fixed col
                nc.vector.tensor_copy(out=onesv, in_=ones_src)
            for t in range(16):
                src, wi = t // 8, t % 8
                ps = psum.tile([P, C], F32, tag=f"q{t % 2}", name="ps_hpa")
                for o in range(2):
                    nc.tensor.matmul(
                        ps[:],
                        hbnF[:, o, src, wi * P : (wi + 1) * P],
                        Wh_sb[:, o, :],
                        start=(o == 0), stop=(o == 1),
                    )
                psv = ps.rearrange("p (a e d) -> p a e d", a=2, e=2)
                nc.vector.tensor_copy(out=hpA[:, t, :, 0:64], in_=psv[:, :, 0, :])
                nc.vector.tensor_copy(out=hpA[:, t, :, 65:129], in_=psv[:, :, 1, :])

            # ---------- phase F: attention per head ----------
            g_ext = ppool.tile([P, 2, R + 2], F32, tag="g_ext")
            den4 = ppool.tile([4, 2 * 512], F32, tag="den4")
            aggT = ppool.tile([P, 2, R], F32, tag="aggT")

            for hh in range(2):
                for w in range(2):
                    aggE = psum.tile([P, 512], F32, tag="aggE", name="aggE")
                    aggO = psum.tile([P, 512], F32, tag="aggO", name="aggO")
                    for t in range(16):
                        egE = psum.tile(
                            [P, 512], F32, tag=f"q{t % 2}", name="egE"
                        )
                        egO = psum.tile(
                            [P, 512], F32, tag=f"q{2 + t % 2}", name="egO"
                        )
                        # even/odd head e-matmuls adjacent: lhsT partition
                        # bases 0/64 -> tile_position row-packing, concurrent
                        nc.tensor.matmul(
                            egE[:],
                            hpTf[0:64, hh, t * P : (t + 1) * P],
                            hpTo[0:64, hh, w * 512 : (w + 1) * 512],
                            start=True, stop=True,
                        )
                        nc.tensor.matmul(
                            egO[:],
                            hpTf[64:128, hh, t * P : (t + 1) * P],
                            hpTo[64:128, hh, w * 512 : (w + 1) * 512],
                            start=True, stop=True,
                        )
                        # leaky-relu in place on PSUM (keeps f32 precision),
                        # one custom DVE op each
                        nc.vector._custom_dve(
                            LRELU_ANT, out=egE[:], in0=egE[:], s0=SLOPE
                        )
                        nc.vector._custom_dve(
                            LRELU_ANT, out=egO[:], in0=egO[:], s0=SLOPE
                        )
                        elE = expool.tile([P, 512], F32R, tag="elE", name="elE")
                        elO = expool.tile([P, 512], F32R, tag="elO", name="elO")
                        nc.scalar.activation(
                            out=elE[:], in_=egE[:], func=AF.Exp, bias=cm40[:]
                        )
                        nc.scalar.activation(
                            out=elO[:], in_=egO[:], func=AF.Exp, bias=cm40[:]
                        )
                        nc.tensor.matmul(
                            aggE[0:65, :], hpA[:, t, hh, 0:65], elE[:],
                            start=(t == 0), stop=(t == 15),
                        )
                        nc.tensor.matmul(
                            aggO[0:65, :], hpA[:, t, hh, 65:130], elO[:],
                            start=(t == 0), stop=(t == 15),
                        )
                    for eo, agg in ((0, aggE), (1, aggO)):
                        h = 2 * hh + eo
                        dstage = wpool.tile(
                            [P, 512], F32, tag="stage", name="dstage"
                        )
                        nc.vector.tensor_copy(
                            out=dstage[64:65, :], in_=agg[64:65, :]
                        )
                        nc.sync.dma_start(
                            den4[h : h + 1, w * 512 : (w + 1) * 512],
                            dstage[64:65, :],
                        )
                        if eo == 0:
                            nc.scalar.activation(
                                out=aggT[0:64, hh, w * 512 : (w + 1) * 512],
                                in_=agg[0:64, :], func=AF.Copy,
                            )
                        else:
                            astage = wpool.tile(
                                [P, 512], F32, tag="stage", name="astage"
                            )
                            nc.scalar.activation(
                                out=astage[0:64, :], in_=agg[0:64, :],
                                func=AF.Copy,
                            )
                            nc.sync.dma_start(
                                aggT[64:128, hh, w * 512 : (w + 1) * 512],
                                astage[0:64, :],
                            )
            # recip4 = alpha_gat / den  (DVE approx reciprocal, ~2 ULP)
            rec4f = ppool.tile([4, 2 * 512], F32, tag="rec4f")
            rscr = ppool.tile([4, 2 * 512], F32, tag="rscr")
            nc.vector.reciprocal_approx_accurate(
                out=rec4f[:], in_=den4[:], scratch=rscr[:]
            )
            nc.vector.tensor_scalar_mul(rec4f[:], rec4f[:], float(alpha_gat))
            rec4 = ppool.tile([4, 2 * 512], F32R, tag="rec4")
            nc.vector.tensor_copy(out=rec4[:], in_=rec4f[:])
            # g = aggT * recip_bc + (1-alpha)*hbnT   -> g_ext[:, :, 1:R+1]
            for o in range(2):
                for w in range(2):
                    bc = psum.tile([P, 512], F32, tag="q2", name="bc")
                    nc.tensor.matmul(
                        bc[:],
                        Ebc_sb[:, o * P : (o + 1) * P],
                        rec4[:, w * 512 : (w + 1) * 512],
                        start=True, stop=True,
                    )
                    gsl = g_ext[:, o, 1 + w * 512 : 1 + (w + 1) * 512]
                    nc.vector.tensor_tensor(
                        gsl, aggT[:, o, w * 512 : (w + 1) * 512], bc[:], ALU.mult
                    )
                    from concourse import dve_ops
                    nc.vector._custom_dve(
                        dve_ops.AFFINE_THEN_ADD,
                        out=gsl,
                        in0=hbnT[:, o, w * 512 : (w + 1) * 512],
                        in1=gsl,
                        s0=float(1.0 - alpha_gat),
                        s1=0.0,
                    )

            if debug_taps:
                nc.sync.dma_start(taps["d_hT"][:, :, :], hT[:])
                nc.sync.dma_start(taps["d_hbnT"][:, :, :], hbnT[:])
                hpTo_f = wpool.tile([P, 2, R], F32, tag="hpTo_f")
                nc.vector.tensor_copy(out=hpTo_f[:], in_=hpTo[:])
                nc.sync.dma_start(taps["d_hpTo"][:, :, :], hpTo_f[:])
                nc.sync.dma_start(taps["d_aggT"][:, :, :], aggT[:])
                nc.sync.dma_start(taps["d_den4"][:, :], den4[:])
                rec4_f = wpool.tile([4, 2 * 512], F32, tag="rec4_f")
                nc.vector.tensor_copy(out=rec4_f[:], in_=rec4[:])
                nc.sync.dma_start(taps["d_rec4"][:, :], rec4_f[:])
                nc.sync.dma_start(taps["d_st1"][:, :], st1[:])
                nc.sync.dma_start(taps["d_stg1"][:, :], stg1[:])

            # ---------- phase G: halo exchange of g boundary columns
            # cc3_in cols: [first o0, first o1, last o0, last o1]
            nc.sync.dma_start(
                cc3_in.rearrange("p (f o) -> p f o", f=2)[:, 0, :], g_ext[:, :, 1]
            )
            nc.sync.dma_start(
                cc3_in.rearrange("p (f o) -> p f o", f=2)[:, 1, :], g_ext[:, :, R]
            )
            nc.gpsimd.collective_compute(
                "AllGather", ALU.bypass, replica_groups=PAIRS,
                ins=[cc3_in.opt()], outs=[cc3_out.opt()],
            )
            hal = ppool.tile([P, 2, 2, 2], F32, tag="hal")  # [p, src, f/l, o]
            for src in range(2):
                nc.sync.dma_start(
                    hal[:, src, :, :],
                    cc3_out[src].rearrange("p (f o) -> p f o", f=2),
                )
            # halo[L/R][p, o] = sum_{src, fl} hal[p, src, fl, o] * hmask[LR, src, fl]
            halv = bass.AP(
                tensor=hal.tensor, offset=hal.offset,
                ap=[hal.ap[0], [1, 2], [4, 2], [2, 2]],
            )  # [p, o, src, fl]
            for lr, col in ((0, 0), (1, R + 1)):
                mv = bass.AP(
                    tensor=hm_sb.tensor, offset=hm_sb.offset + lr * 4,
                    ap=[hm_sb.ap[0], [0, 2], [2, 2], [1, 2]],
                )  # [p, o(bc), src, fl]
                tmp = wpool.tile([P, 2, 2, 2], F32, tag="haltmp")
                nc.vector.tensor_tensor(tmp[:], halv, mv, ALU.mult)
                nc.vector.reduce_sum(g_ext[:, :, col], tmp[:], axis=AX.XY)

            # ---------- phase H: TCN conv ----------
            gr_ext = ppool.tile([P, 2, R + 2], F32R, tag="gr_ext")
            nc.vector.tensor_copy(out=gr_ext[:], in_=g_ext[:])
            tconv = ppool.tile([P, 2, R], F32, tag="hT_share", name="tconv")
            for oo in range(2):
                for w in range(2):
                    ps = psum.tile([P, 512], F32, tag=f"q{(oo * 2 + w) % 2}", name="ps_cv")
                    first = True
                    for oi in range(2):
                        for k in range(3):
                            nc.tensor.matmul(
                                ps[:],
                                Wk_sb[:, oi, k, oo * P : (oo + 1) * P],
                                gr_ext[:, oi, w * 512 + k : w * 512 + k + 512],
                                start=first, stop=(oi == 1 and k == 2),
                            )
                            first = False
                    nc.vector.tensor_copy(
                        out=tconv[:, oo, w * 512 : (w + 1) * 512], in_=ps[:]
                    )

            if debug_taps:
                nc.sync.dma_start(taps["d_g"][:, :, :], g_ext[:])
                nc.sync.dma_start(taps["d_tconv"][:, :, :], tconv[:])

            # ---------- phase I: BN2 + residual + output ----------
            st2 = ppool.tile([P, 4], F32, tag="st2")
            sq2 = wpool.tile([P, R], F32, tag="sq_scr", name="sq2")
            for o in range(2):
                nc.vector.reduce_sum(st2[:, o : o + 1], tconv[:, o, :], axis=AX.X)
                nc.scalar.activation(
                    out=sq2[:], in_=tconv[:, o, :], func=AF.Square,
                    accum_out=st2[:, 2 + o : 3 + o],
                )
            nc.sync.dma_start(cc4_in[:, :], st2[:])
            nc.gpsimd.collective_compute(
                "AllReduce", ALU.add, replica_groups=ALL8,
                ins=[cc4_in.opt()], outs=[cc4_out.opt()],
            )
            stg2 = ppool.tile([P, 4], F32, tag="stg2")
            nc.sync.dma_start(stg2[:], cc4_out[:, :])
            fold = alpha_tcn if alpha_tcn > 0 else 1.0
            A2, C2 = bn_affine(stg2, g2_sb, b2_sb, "bn2", fold=fold)

            final = ppool.tile([P, 2, R], F32, tag="final")
            from concourse import dve_ops
            for o in range(2):
                nc.scalar.activation(
                    out=final[:, o, :], in_=tconv[:, o, :], func=AF.Relu,
                    scale=A2[:, o : o + 1], bias=C2[:, o : o + 1],
                )
                if fold != alpha_tcn:  # alpha_tcn <= 0: scale separately
                    nc.vector.tensor_scalar_mul(
                        final[:, o, :], final[:, o, :], float(alpha_tcn)
                    )
                nc.vector._custom_dve(
                    dve_ops.AFFINE_THEN_ADD,
                    out=final[:, o, :],
                    in0=g_ext[:, o, 1 : R + 1],
                    in1=final[:, o, :],
                    s0=float(1.0 - alpha_tcn),
                    s1=0.0,
                )
                nc.sync.dma_start(out[:, o, :], final[:, o, :])

    nc.compile()
    return nc


def _f32r(a):
    """Round f32 to the fp32r grid (11-bit mantissa) so DMA'd data matches
    what the PE consumes; lets F32R DRAM tensors skip casting DMAs."""
    a = np.ascontiguousarray(a, np.float32)
    b = a.view(np.uint32).astype(np.uint64)
    b = ((b + 0x800) & 0xFFFFF000).astype(np.uint32)
    return b.view(np.float32)


def _prep_inputs(x, adj, W_sage, b_sage, bn1_gamma, bn1_beta, Wh,
                 conv_w, bn2_gamma, bn2_beta):
    """Build the 8 per-core input maps (host-side numpy)."""
    x = np.asarray(x, np.float32)
    adj = np.asarray(adj, np.float32)
    Whp = np.ascontiguousarray(
        np.asarray(Wh, np.float32).transpose(1, 0, 2).reshape(C, H * DH)
    )
    WkT = np.ascontiguousarray(np.asarray(conv_w, np.float32).transpose(2, 1, 0))
    Ebc = np.zeros((4, C), np.float32)
    for c in range(C):
        Ebc[(c % P) // 64 + 2 * (c // P), c] = 1.0

    shared = dict(
        W=_f32r(np.asarray(W_sage, np.float32)),
        bs=np.asarray(b_sage, np.float32),
        g1=np.asarray(bn1_gamma, np.float32),
        b1=np.asarray(bn1_beta, np.float32),
        Whp=_f32r(Whp), WkT=_f32r(WkT),
        g2=np.asarray(bn2_gamma, np.float32),
        b2=np.asarray(bn2_beta, np.float32),
        Ebc=_f32r(Ebc),
    )
    in_maps = []
    for core in range(NC):
        b, s = core // 2, core % 2
        hmask = np.zeros((2, 2, 2), np.float32)  # [L/R, src, first/last]
        if s == 0:
            hmask[1, 1, 0] = 1.0  # right halo = partner(rank1) first col
        else:
            hmask[0, 0, 1] = 1.0  # left halo = partner(rank0) last col
        m = dict(
            xT=_f32r(x[b].T),
            adjTc=_f32r(adj[s * R : (s + 1) * R, :].T),
            hmask=hmask,
            **shared,
        )
        in_maps.append(m)
    return in_maps


def _assemble(results):
    out = np.empty((B, N, C), np.float32)
    for core in range(NC):
        b, s = core // 2, core % 2
        r = results[core]["out"]  # [P, 2, R]
        out[b, s * R : (s + 1) * R, :] = r.transpose(2, 1, 0).reshape(R, C)
    return out


_CACHE = {}


def kernel(x, adj, W_sage, b_sage, bn1_gamma, bn1_beta, Wh, alpha_gat,
           conv_w, conv_b, bn2_gamma, bn2_beta, alpha_tcn, **_unused):
    ag, at = float(alpha_gat), float(alpha_tcn)
    key = (ag, at)
    if key not in _CACHE:
        _CACHE[key] = build_program(ag, at)
    nc = _CACHE[key]
    in_maps = _prep_inputs(x, adj, W_sage, b_sage, bn1_gamma, bn1_beta, Wh,
                           conv_w, bn2_gamma, bn2_beta)
    res = run_bass_kernel_spmd(nc, in_maps, core_ids=list(range(NC)))
    return _assemble(res.results)


if __name__ == "__main__":
    import sys
    sys.path.insert(0, "/root/problem")
    import reference
    inputs = {k: np.asarray(v) for k, v in reference.setup_inputs().items()}
    expected = np.asarray(reference.reference(**inputs))
    actual = kernel(**inputs)
    err = np.abs(actual - expected)
    rel = np.linalg.norm(actual - expected) / np.linalg.norm(expected)
    print("max abs err:", err.max(), "rel:", rel)



# revision 24
# speedup vs baseline: 1.7095x; 1.7095x over previous
"""Trainium2 Bass kernel for nn_GCNWithMultiHeadGATAndTCN_42356967473538.

Sharding: 8 cores = (batch b in 0..3) x (node-half s in 0..1).
Each core computes the FULL batch-b pipeline through BN1 + projections
(redundantly within a pair) so that no activation exchange is needed;
only its own 1024+2 query columns go through attention / TCN / output.

Per-core node axis is ROTATED so that own nodes sit at columns 1..1024
with halo columns 0 and 1025 (edge-masked per core via `emask` input).
This makes the SPMD instruction stream core-uniform; all per-core
differences live in the input data (xT/adjT rotation, emask).

Cross-core communication: only two tiny stats AllGathers ([P,4] f32,
all 8 cores) for the training-mode BatchNorm moments (bn1, bn2); each
core reduces the gathered 8 copies locally.

dtypes: fp16 for x/adj/weights/activations (same 11-bit mantissa as
f32r), f32r for exp/attention values (range), f32 accumulation in PSUM.
"""

import numpy as np

import concourse.bass as bass
import concourse.mybir as mybir
import concourse.tile as tile
from concourse import bacc, dve_ops
from concourse.bass_utils import run_bass_kernel_spmd
from concourse.dve_spec import Spec, Src0, C0, maxx, lower, _has_src1
from concourse.dve_uop import DveOpSpec


def _register_lrelu_op():
    """Custom single-pass DVE leaky-relu: out = max(in0, in0*s0)."""
    if "LRELU_ANT" in dve_ops._SUB_OPCODE_FOR_NAME:
        return dve_ops.CUSTOM_DVE_SPECS and next(
            op for op in dve_ops.OPS if op.name == "LRELU_ANT"
        )
    spec = Spec(
        body=maxx(Src0, Src0 * C0),
        reference=lambda in0, in1, s0, s1, imm2: np.maximum(
            np.nan_to_num(in0, nan=0.0, posinf=np.inf, neginf=-np.inf),
            in0 * s0,
        ).astype(np.float32),
    )
    row = dve_ops._CUSTOM_DVE_ROW_BASE + len(dve_ops.OPS)
    assert row < 0x20
    shas = {}
    for ver in ("v3", "v4"):
        try:
            tmp = DveOpSpec(name="LRELU_ANT", opcode=row, uops=lower(spec, ver=ver),
                            rd1_en=_has_src1(spec))
            shas[ver] = tmp.sha(ver)
        except Exception:
            pass
    op = dve_ops.DveOp("LRELU_ANT", spec, False, shas)
    dve_ops.OPS.append(op)
    dve_ops.CUSTOM_DVE_SPECS["LRELU_ANT"] = spec
    dve_ops._SUB_OPCODE_FOR_NAME["LRELU_ANT"] = row
    return op


LRELU_ANT = _register_lrelu_op()

F32 = mybir.dt.float32
F32R = mybir.dt.float32r
F16 = mybir.dt.float16
AF = mybir.ActivationFunctionType
ALU = mybir.AluOpType
AX = mybir.AxisListType

B, N, FEAT, C, H, DH = 4, 2048, 256, 256, 4, 64
P = 128
R = N // 2            # own nodes per core (1024)
Q = R + 2             # query columns incl. both halos (1026)
NC = 8                # cores
EPS = 1e-5
SLOPE = 0.2
EXP_SHIFT = 64.0      # softmax-invariant shift keeps exp in f32 range
CNT1 = float(2 * B * N)   # bn1 sample count x2 (pairs duplicate batches)
CNT2 = float(B * N)       # bn2 sample count (own node halves, no dup)

ALL8 = [list(range(NC))]

# which lrelu tiles run on the Pool engine (rest on DVE)
POOL_T = frozenset({1, 3, 5, 7, 9, 11, 13})


def _bc_ap(ap, parts=P):
    """Broadcast a DRAM AP across `parts` partitions (stride-0 partition dim)."""
    return bass.AP(tensor=ap.tensor, offset=ap.offset, ap=[[0, parts], *ap.ap])


def build_program(alpha_gat: float, alpha_tcn: float, sim_safe: bool = False,
                  **_unused):
    nc = bacc.Bacc(
        "TRN2", target_bir_lowering=False, debug=False, num_devices=NC
    )

    def din(name, shape, dt=F32):
        return nc.dram_tensor(name, shape, dt, kind="ExternalInput").ap()

    xT = din("xT", [FEAT, N], F16)       # x[b].T, node-rotated
    adjT = din("adjT", [N, N], F16)      # adj.T, node-rotated both axes
    Wp = din("Wp", [FEAT, C], F16)
    bs = din("bs", [C])
    g1 = din("g1", [C])
    b1 = din("b1", [C])
    WhT = din("WhT", [C, 2 * P], F16)    # cols = hh*128 + eo*64 + d
    WkT = din("WkT", [3, C, C], F16)     # conv_w[:, :, k].T -> [k, cin, cout]
    cb = din("cb", [C])
    g2 = din("g2", [C])
    b2 = din("b2", [C])
    Ebc = din("Ebc", [4, C], F32R)       # head->channel one-hot (recip bcast)
    emask = din("emask", [2])            # halo-col validity [left, right]

    out = nc.dram_tensor("out", [P, 2, R], F32, kind="ExternalOutput").ap()

    # internal DRAM bounce buffers for the stats collectives
    cc1_in = nc.dram_tensor("cc1_in", [P, 4], F32).ap()
    cc1_out = nc.dram_tensor("cc1_out", [NC, P, 4], F32).ap()
    cc2_in = nc.dram_tensor("cc2_in", [P, 4], F32).ap()
    cc2_out = nc.dram_tensor("cc2_out", [NC, P, 4], F32).ap()

    with tile.TileContext(nc) as tc:
        with (
            tc.tile_pool(name="persist", bufs=1) as ppool,
            tc.tile_pool(name="work", bufs=2) as wpool,
            tc.tile_pool(name="adjp", bufs=4) as adjpool,
            tc.tile_pool(name="expp", bufs=2) as expool,
            tc.tile_pool(name="psum", bufs=1, space="PSUM") as psum,
        ):
            # ---------- constants ----------
            Wp_sb = ppool.tile([P, 2, C], F16, tag="Wp_sb")
            nc.sync.dma_start(Wp_sb[:], Wp.rearrange("(o p) c -> p o c", p=P))
            xT_sb = ppool.tile([P, 2, N], F16, tag="xT_sb")
            xTv = xT.rearrange("(ko p) m -> p ko m", p=P)
            nc.sync.dma_start(xT_sb[:, :, 0:N // 2], xTv[:, :, 0:N // 2])
            nc.sync.dma_start(xT_sb[:, :, N // 2:N], xTv[:, :, N // 2:N])
            # prefetch the first adj chunks before the remaining constants so
            # the (in-order) DMA queue feeds phase B without head-of-line
            # stalls; the rest are issued in the B loop (bufs=4 rotation).
            adj_tiles = []
            for t in range(4):
                at = adjpool.tile([P, N], F16, tag="at", name=f"at{t}")
                nc.sync.dma_start(at[:], adjT[t * P:(t + 1) * P, :])
                adj_tiles.append(at)
            Wh_sb = ppool.tile([P, 2, 2, P], F16, tag="Wh_sb")
            nc.sync.dma_start(
                Wh_sb[:], WhT.rearrange("(o p) c -> p o c", p=P)
            )
            Wk_sb = ppool.tile([P, 2, 3, C], F16, tag="Wk_sb")
            for k in range(3):
                nc.sync.dma_start(
                    Wk_sb[:, :, k, :],
                    WkT[k].rearrange("(o p) c -> p o c", p=P),
                )
            bs_sb = ppool.tile([P, 2], F32, tag="bs_sb")
            nc.sync.dma_start(bs_sb[:], bs.rearrange("(o p) -> p o", p=P))
            g1_sb = ppool.tile([P, 2], F32, tag="g1_sb")
            nc.sync.dma_start(g1_sb[:], g1.rearrange("(o p) -> p o", p=P))
            b1_sb = ppool.tile([P, 2], F32, tag="b1_sb")
            nc.sync.dma_start(b1_sb[:], b1.rearrange("(o p) -> p o", p=P))
            cb_sb = ppool.tile([P, 2], F32, tag="cb_sb")
            nc.sync.dma_start(cb_sb[:], cb.rearrange("(o p) -> p o", p=P))
            g2_sb = ppool.tile([P, 2], F32, tag="g2_sb")
            nc.sync.dma_start(g2_sb[:], g2.rearrange("(o p) -> p o", p=P))
            b2_sb = ppool.tile([P, 2], F32, tag="b2_sb")
            nc.sync.dma_start(b2_sb[:], b2.rearrange("(o p) -> p o", p=P))
            Ebc_sb = ppool.tile([4, C], F32R, tag="Ebc_sb")
            nc.sync.dma_start(Ebc_sb[:], Ebc[:, :])
            em_sb = ppool.tile([P, 2], F32, tag="em_sb")
            nc.sync.dma_start(em_sb[:], _bc_ap(emask[:]))
            cm40 = ppool.tile([P, 1], F32, tag="cm40")
            nc.vector.memset(cm40[:], -EXP_SHIFT)

            # PE warm-up: harmless matmuls on the weight tile so the p-state
            # ramp completes during the input DMAs.
            warm_ps = psum.tile([P, C], F32, tag="q2", name="warm_ps")
            for _ in range(14):
                nc.tensor.matmul(warm_ps[:], Wp_sb[:, 0, 0:P], Wp_sb[:, 0, :],
                                 start=True, stop=True)
            warm_scr = ppool.tile([P, 1], F32, tag="warm_scr")
            nc.vector.reduce_sum(warm_scr[:], warm_ps[:, 0:4], axis=AX.X)

            # ---------- phase A: support = x @ W  [m-part, t, c] ----------
            support = ppool.tile([P, 16, C], F16, tag="support")
            for t in range(16):
                ps = psum.tile([P, C], F32, tag=f"q{t % 2}", name="ps_sup")
                for ko in range(2):
                    nc.tensor.matmul(
                        ps[:], xT_sb[:, ko, t * P:(t + 1) * P], Wp_sb[:, ko, :],
                        start=(ko == 0), stop=(ko == 1),
                    )
                if t % 2 == 0:
                    nc.scalar.activation(out=support[:, t, :], in_=ps[:],
                                         func=AF.Copy)
                else:
                    nc.vector.tensor_copy(out=support[:, t, :], in_=ps[:])

            # ---------- phase B: hT = relu(support^T @ adjT + b)  [c, n] ----
            hT = ppool.tile([P, 2, N], F16, tag="hT")
            st1 = ppool.tile([P, 2, 4], F32, tag="st1")
            PSB_TAGS = ["q0", "q1", "q2", "q3", "aggE", "aggO", "x0", "x1"]
            ps_b = [
                [
                    psum.tile([P, 512], F32, tag=PSB_TAGS[o * 4 + w4],
                              name=f"ps_b{o}{w4}")
                    for w4 in range(4)
                ]
                for o in range(2)
            ]
            st1sq = ppool.tile([P, 2, 4], F32, tag="st1sq")
            sqscr = ppool.tile([P, N], F16, tag="sqscr")
            for t in range(16):
                if t < 4:
                    at = adj_tiles[t]
                else:
                    at = adjpool.tile([P, N], F16, tag="at")
                    nc.sync.dma_start(at[:], adjT[t * P:(t + 1) * P, :])
                for o in range(2):
                    for w4 in range(4):
                        nc.tensor.matmul(
                            ps_b[o][w4][:],
                            support[:, t, o * P:(o + 1) * P],
                            at[:, w4 * 512:(w4 + 1) * 512],
                            start=(t == 0), stop=(t == 15),
                        )
            for o in range(2):
                for w4 in range(4):
                    sl = slice(w4 * 512, (w4 + 1) * 512)
                    nc.scalar.activation(
                        out=hT[:, o, sl],
                        in_=ps_b[o][w4][:],
                        func=AF.Relu,
                        bias=bs_sb[:, o:o + 1],
                        accum_out=st1[:, o, w4:w4 + 1],
                    )
                    # sum of squares on DVE, in parallel with the ACT pass
                    nc.vector.affine_mul_reduce(
                        out=sqscr[:, sl], accum_out=st1sq[:, o, w4:w4 + 1],
                        in0=hT[:, o, sl], in1=hT[:, o, sl],
                        scale=1.0, bias=0.0,
                    )

            # ---------- phase C: BN1 stats + allgather + apply ----------
            stp1 = ppool.tile([P, 4], F32, tag="stp1")
            for o in range(2):
                nc.vector.reduce_sum(stp1[:, o:o + 1], st1[:, o, :], axis=AX.X)
                nc.vector.reduce_sum(
                    stp1[:, 2 + o:3 + o], st1sq[:, o, :], axis=AX.X
                )
            nc.sync.dma_start(cc1_in[:, :], stp1[:])
            nc.gpsimd.collective_compute(
                "AllGather", ALU.bypass, replica_groups=ALL8,
                ins=[cc1_in.opt()], outs=[cc1_out.opt()],
            )
            warm2 = psum.tile([P, C], F32, tag="q2", name="warm2")
            for _ in range(60):
                nc.tensor.matmul(warm2[:], Wp_sb[:, 0, 0:P], Wp_sb[:, 0, :],
                                 start=True, stop=True)
            warm2_scr = ppool.tile([P, 1], F32, tag="warm_scr", name="w2scr")
            nc.vector.reduce_sum(warm2_scr[:], warm2[:, 0:4], axis=AX.X)
            stg1g = ppool.tile([P, NC, 4], F32, tag="stg1g")
            nc.sync.dma_start(
                stg1g[:],
                bass.AP(tensor=cc1_out.tensor, offset=cc1_out.offset,
                        ap=[[4, P], [P * 4, NC], [1, 4]]),
            )
            stg1 = ppool.tile([P, 4], F32, tag="stg1")
            for c4 in range(4):
                nc.vector.reduce_sum(
                    stg1[:, c4:c4 + 1], stg1g[:, :, c4], axis=AX.X
                )

            def bn_affine(stg, gam, bet, tagp, cnt, fold=1.0):
                """A, C with y = relu(x*A + C) == relu(fold*bn(x))."""
                mean = ppool.tile([P, 2], F32, tag=f"{tagp}_mean")
                nc.vector.tensor_scalar_mul(mean[:], stg[:, 0:2], 1.0 / cnt)
                ex2 = ppool.tile([P, 2], F32, tag=f"{tagp}_ex2")
                nc.vector.tensor_scalar_mul(ex2[:], stg[:, 2:4], 1.0 / cnt)
                var = ppool.tile([P, 2], F32, tag=f"{tagp}_var")
                nc.vector.tensor_tensor(var[:], mean[:], mean[:], ALU.mult)
                nc.vector.tensor_tensor(var[:], ex2[:], var[:], ALU.subtract)
                rstd = ppool.tile([P, 2], F32, tag=f"{tagp}_rstd")
                nc.vector.tensor_scalar_add(var[:], var[:], EPS)
                nc.scalar.activation(rstd[:], var[:], AF.Ln)
                nc.scalar.activation(rstd[:], rstd[:], AF.Exp, scale=-0.5)
                A = ppool.tile([P, 2], F32, tag=f"{tagp}_A")
                nc.vector.tensor_tensor(A[:], gam[:], rstd[:], ALU.mult)
                Cc = ppool.tile([P, 2], F32, tag=f"{tagp}_C")
                nc.vector.tensor_tensor(Cc[:], mean[:], A[:], ALU.mult)
                nc.vector.tensor_tensor(Cc[:], bet[:], Cc[:], ALU.subtract)
                if fold != 1.0:
                    nc.vector.tensor_scalar_mul(A[:], A[:], fold)
                    nc.vector.tensor_scalar_mul(Cc[:], Cc[:], fold)
                return A, Cc

            A1, C1 = bn_affine(stg1, g1_sb, b1_sb, "bn1", CNT1)
            hbn = ppool.tile([P, 2, N], F16, tag="hbn")
            for w4 in range(4):
                for o in range(2):
                    sl = slice(w4 * 512, (w4 + 1) * 512)
                    nc.scalar.activation(
                        out=hbn[:, o, sl], in_=hT[:, o, sl], func=AF.Relu,
                        scale=A1[:, o:o + 1], bias=C1[:, o:o + 1],
                    )

            # ---------- phase E: hp projections ----------
            # hpTf: [d-major (eo*64+d), hh, n]  (fp16, e-matmul operands)
            hpTf = ppool.tile([P, 2, N], F16, tag="hpTf")
            for hh in range(2):
                for w4 in range(4):
                    ps = psum.tile([P, 512], F32, tag=f"q{(hh * 4 + w4) % 2}",
                                   name="ps_hpf")
                    for o in range(2):
                        nc.tensor.matmul(
                            ps[:],
                            Wh_sb[:, o, hh, :],
                            hbn[:, o, w4 * 512:(w4 + 1) * 512],
                            start=(o == 0), stop=(o == 1),
                        )
                    nc.vector.tensor_copy(
                        out=hpTf[:, hh, w4 * 512:(w4 + 1) * 512], in_=ps[:]
                    )
            # hpA: node-major [n-part, t, hh, 130] f32r with ones cols 64/129
            hpA = ppool.tile([P, 16, 2, 130], F32R, tag="hpA")
            ones1 = ppool.tile([P, 1], F32, tag="ones1")
            nc.vector.memset(ones1[:], 1.0)
            ones_src = bass.AP(
                tensor=ones1.tensor, offset=ones1.offset,
                ap=[ones1.ap[0], [0, 16], [0, 2]],
            )
            for col in (64, 129):
                onesv = bass.AP(
                    tensor=hpA.tensor, offset=hpA.offset + col,
                    ap=[hpA.ap[0], [260, 16], [130, 2]],
                )  # [p, t, hh] at fixed col
                nc.vector.tensor_copy(out=onesv, in_=ones_src)
            for t in range(16):
                ps = psum.tile([P, C], F32, tag=f"q{2 + t % 2}", name="ps_hpa")
                for o in range(2):
                    nc.tensor.matmul(
                        ps[:],
                        hbn[:, o, t * P:(t + 1) * P],
                        Wh_sb[:, o, :, :].rearrange("p hh x -> p (hh x)"),
                        start=(o == 0), stop=(o == 1),
                    )
                # psum cols (hh, eo, d) -> hpA[:, t, hh, eo*65 + d]
                psv = ps.rearrange("p (hh eo d) -> p hh eo d", hh=2, eo=2)
                dst = bass.AP(
                    tensor=hpA.tensor, offset=hpA.offset + t * 260,
                    ap=[hpA.ap[0], [130, 2], [65, 2], [1, 64]],
                )
                if t % 2 == 0:
                    nc.scalar.activation(out=dst, in_=psv[:], func=AF.Copy)
                else:
                    nc.vector.tensor_copy(out=dst, in_=psv[:])

            # ---------- phase F: attention fused with g/conv/bn2-stats ------
            aggT = ppool.tile([P, 2, Q], F32, tag="aggT")
            den4 = ppool.tile([4, Q], F32, tag="den4")
            rec4f = ppool.tile([4, Q], F32, tag="rec4f")
            rscr = ppool.tile([4, Q], F32, tag="rscr")
            rec4 = ppool.tile([4, Q], F32R, tag="rec4")
            g = ppool.tile([P, 2, Q], F16, tag="g")
            tconv = ppool.tile([P, 2, R], F16, tag="tconv")
            st2 = ppool.tile([P, 2, 3], F32, tag="st2")
            st2sq = ppool.tile([P, 2, 3], F32, tag="st2sq")
            emv = bass.AP(
                tensor=em_sb.tensor, offset=em_sb.offset,
                ap=[em_sb.ap[0], [0, 2], [1, 1]],
            )  # [p, o(bc), 1] at left-mask col; offset +1 for right

            def attn_block(hh, qb):
                """e/lrelu/exp/agg for head pair hh, query cols [qb, qb+512)."""
                aggE = psum.tile([P, 512], F32, tag="aggE", name="aggE")
                aggO = psum.tile([P, 512], F32, tag="aggO", name="aggO")
                for g4 in range(4):
                    elE = expool.tile([P, 4, 512], F32R, tag="elE")
                    elO = expool.tile([P, 4, 512], F32R, tag="elO")
                    for tt in range(4):
                        t = g4 * 4 + tt
                        egE = psum.tile([P, 512], F32, tag=f"q{t % 2}",
                                        name="egE")
                        egO = psum.tile([P, 512], F32, tag=f"q{2 + t % 2}",
                                        name="egO")
                        nc.tensor.matmul(
                            egE[:],
                            hpTf[0:64, hh, t * P:(t + 1) * P],
                            hpTf[0:64, hh, qb:qb + 512],
                            start=True, stop=True,
                        )
                        nc.tensor.matmul(
                            egO[:],
                            hpTf[64:128, hh, t * P:(t + 1) * P],
                            hpTf[64:128, hh, qb:qb + 512],
                            start=True, stop=True,
                        )
                        for eo, eg, el in ((0, egE, elE), (1, egO, elO)):
                            nc.vector._custom_dve(
                                LRELU_ANT, out=el[:, tt, :], in0=eg[:],
                                s0=SLOPE,
                            )
                    nc.scalar.activation(out=elE[:], in_=elE[:],
                                         func=AF.Exp, bias=cm40[:])
                    nc.scalar.activation(out=elO[:], in_=elO[:],
                                         func=AF.Exp, bias=cm40[:])
                    for tt in range(4):
                        t = g4 * 4 + tt
                        nc.tensor.matmul(
                            aggE[0:65, :], hpA[:, t, hh, 0:65],
                            elE[:, tt, :],
                            start=(t == 0), stop=(t == 15),
                        )
                        nc.tensor.matmul(
                            aggO[0:65, :], hpA[:, t, hh, 65:130],
                            elO[:, tt, :],
                            start=(t == 0), stop=(t == 15),
                        )
                # drain agg (psum -> sbuf stage -> aggT/den rows via DMA)
                stageE = wpool.tile([65, 512], F32, tag="stage", name="stageE")
                stageO = wpool.tile([65, 512], F32, tag="stage", name="stageO")
                nc.vector.tensor_copy(out=stageE[:], in_=aggE[0:65, :])
                nc.vector.tensor_copy(out=stageO[:], in_=aggO[0:65, :])
                nc.sync.dma_start(aggT[0:64, hh, qb:qb + 512], stageE[0:64, :])
                nc.sync.dma_start(
                    den4[2 * hh:2 * hh + 1, qb:qb + 512], stageE[64:65, :]
                )
                nc.sync.dma_start(aggT[64:128, hh, qb:qb + 512], stageO[0:64, :])
                nc.sync.dma_start(
                    den4[2 * hh + 1:2 * hh + 2, qb:qb + 512], stageO[64:65, :]
                )

            def g_chunk(qb, ws):
                """recip + g assembly for query cols [qb, qb+ws)."""
                nc.vector.reciprocal_approx_accurate(
                    out=rec4f[:, qb:qb + ws], in_=den4[:, qb:qb + ws],
                    scratch=rscr[:, qb:qb + ws],
                )
                nc.vector.tensor_scalar_mul(
                    rec4[:, qb:qb + ws], rec4f[:, qb:qb + ws], float(alpha_gat)
                )
                for o in range(2):
                    bc = psum.tile([P, 512], F32, tag="x0", name="bc")
                    nc.tensor.matmul(
                        bc[:, 0:ws],
                        Ebc_sb[:, o * P:(o + 1) * P],
                        rec4[:, qb:qb + ws],
                        start=True, stop=True,
                    )
                    gsl = g[:, o, qb:qb + ws]
                    nc.vector.tensor_tensor(
                        gsl, aggT[:, o, qb:qb + ws], bc[:, 0:ws], ALU.mult
                    )
                    nc.vector._custom_dve(
                        dve_ops.AFFINE_THEN_ADD,
                        out=gsl,
                        in0=hbn[:, o, qb:qb + ws],
                        in1=gsl,
                        s0=float(1.0 - alpha_gat),
                        s1=0.0,
                    )

            def mask_edge(col, mi):
                """halo-edge validity mask on g col (mi: 0=left, 1=right)."""
                gsl = bass.AP(
                    tensor=g.tensor, offset=g.offset + col,
                    ap=[g.ap[0], [Q, 2], [1, 1]],
                )
                mv = bass.AP(
                    tensor=em_sb.tensor, offset=em_sb.offset + mi,
                    ap=[em_sb.ap[0], [0, 2], [1, 1]],
                )
                nc.vector.tensor_tensor(gsl, gsl, mv, ALU.mult)

            CONV_CHUNKS = ((1, 511), (511, 1021), (1021, 1025))

            def conv_chunk(ci):
                """conv output cols [a, b) -> tconv cols [a-1, b-1) + stats."""
                a, b_ = CONV_CHUNKS[ci]
                width = b_ - a
                for oo in range(2):
                    ps = psum.tile([P, 512], F32, tag=("x1", "x0")[oo], name="ps_cv")
                    first = True
                    for oi in range(2):
                        for k in range(3):
                            nc.tensor.matmul(
                                ps[:, 0:width],
                                Wk_sb[:, oi, k, oo * P:(oo + 1) * P],
                                g[:, oi, a - 1 + k:b_ - 1 + k],
                                start=first, stop=(oi == 1 and k == 2),
                            )
                            first = False
                    # conv_b omitted: bn2 (training mode) follows directly, so
                    # a per-channel constant shift cancels exactly.
                    tsl = tconv[:, oo, a - 1:b_ - 1]
                    nc.scalar.activation(
                        out=tsl, in_=ps[:, 0:width], func=AF.Copy,
                        accum_out=st2[:, oo, ci:ci + 1],
                    )
                    nc.vector.affine_mul_reduce(
                        out=sqscr[:, 0:width],
                        accum_out=st2sq[:, oo, ci:ci + 1],
                        in0=tsl, in1=tsl, scale=1.0, bias=0.0,
                    )

            for w in range(2):
                qb = w * 512
                for hh in range(2):
                    attn_block(hh, qb)
                g_chunk(qb, 512)
                if w == 0:
                    mask_edge(0, 0)
                conv_chunk(w)

            # ----- halo queries (cols 1024, 1025) -----
            # eh_ps/elH cols: hh*64 + eo*32 + t*2 + q
            eh_ps = psum.tile([P, 128], F32, tag="q0", name="eh_ps")
            for hh in range(2):
                for eo in range(2):
                    sl = slice(eo * 64, (eo + 1) * 64)
                    for t in range(16):
                        cbase = hh * 64 + eo * 32 + t * 2
                        nc.tensor.matmul(
                            eh_ps[:, cbase:cbase + 2],
                            hpTf[sl, hh, t * P:(t + 1) * P],
                            hpTf[sl, hh, R:R + 2],
                            start=True, stop=True,
                        )
            elH = expool.tile([P, 128], F32R, tag="elH")
            nc.vector._custom_dve(LRELU_ANT, out=elH[:], in0=eh_ps[:], s0=SLOPE)
            nc.scalar.activation(out=elH[:], in_=elH[:], func=AF.Exp,
                                 bias=cm40[:])
            aggH = psum.tile([P, 4, 2], F32, tag="aggE", name="aggH")
            for hh in range(2):
                for eo in range(2):
                    for t in range(16):
                        cbase = hh * 64 + eo * 32 + t * 2
                        nc.tensor.matmul(
                            aggH[0:65, 2 * hh + eo, :],
                            hpA[:, t, hh, eo * 65:(eo + 1) * 65],
                            elH[:, cbase:cbase + 2],
                            start=(t == 0), stop=(t == 15),
                        )
            stageH = wpool.tile([65, 4, 2], F32, tag="stageH")
            nc.vector.tensor_copy(out=stageH[:], in_=aggH[0:65, :, :])
            nc.sync.dma_start(den4[0:4, R:R + 2], stageH[64:65, :, :])
            for hh in range(2):
                nc.vector.tensor_copy(
                    out=aggT[0:64, hh, R:R + 2], in_=stageH[0:64, 2 * hh, :]
                )
                nc.sync.dma_start(
                    aggT[64:128, hh, R:R + 2], stageH[0:64, 2 * hh + 1, :]
                )

            # ----- halo g + last conv chunk + BN2 stats finalize -----
            g_chunk(R, 2)
            mask_edge(Q - 1, 1)
            conv_chunk(2)

            # ---------- phase I: BN2 + residual + output ----------
            stp2 = ppool.tile([P, 4], F32, tag="stp2")
            for o in range(2):
                nc.vector.reduce_sum(stp2[:, o:o + 1], st2[:, o, :], axis=AX.X)
                nc.vector.reduce_sum(
                    stp2[:, 2 + o:3 + o], st2sq[:, o, :], axis=AX.X
                )
            nc.sync.dma_start(cc2_in[:, :], stp2[:])
            nc.gpsimd.collective_compute(
                "AllGather", ALU.bypass, replica_groups=ALL8,
                ins=[cc2_in.opt()], outs=[cc2_out.opt()],
            )
            stg2g = ppool.tile([P, NC, 4], F32, tag="stg2g")
            nc.sync.dma_start(
                stg2g[:],
                bass.AP(tensor=cc2_out.tensor, offset=cc2_out.offset,
                        ap=[[4, P], [P * 4, NC], [1, 4]]),
            )
            stg2 = ppool.tile([P, 4], F32, tag="stg2")
            for c4 in range(4):
                nc.vector.reduce_sum(
                    stg2[:, c4:c4 + 1], stg2g[:, :, c4], axis=AX.X
                )
            fold = alpha_tcn if alpha_tcn > 0 else 1.0
            A2, C2 = bn_affine(stg2, g2_sb, b2_sb, "bn2", CNT2, fold=fold)

            fin = ppool.tile([P, 2, R], F32, tag="fin")
            HR = R // 2
            for o in range(2):
                for q2 in range(2):
                    sl = slice(q2 * HR, (q2 + 1) * HR)
                    nc.scalar.activation(
                        out=fin[:, o, sl], in_=tconv[:, o, sl], func=AF.Relu,
                        scale=A2[:, o:o + 1], bias=C2[:, o:o + 1],
                    )
                    if fold != alpha_tcn:  # alpha_tcn <= 0: scale separately
                        nc.vector.tensor_scalar_mul(
                            fin[:, o, sl], fin[:, o, sl], float(alpha_tcn)
                        )
                    nc.vector._custom_dve(
                        dve_ops.AFFINE_THEN_ADD,
                        out=fin[:, o, sl],
                        in0=g[:, o, 1 + q2 * HR:1 + (q2 + 1) * HR],
                        in1=fin[:, o, sl],
                        s0=float(1.0 - alpha_tcn),
                        s1=0.0,
                    )
                    nc.sync.dma_start(out[:, o, sl], fin[:, o, sl])

    nc.compile()
    return nc


def _prep_inputs(x, adj, W_sage, b_sage, bn1_gamma, bn1_beta, Wh,
                 conv_w, bn2_gamma, bn2_beta, conv_b=None):
    """Build the 8 per-core input maps (host-side numpy)."""
    x = np.asarray(x, np.float32)
    adj = np.asarray(adj, np.float32)
    Wh = np.asarray(Wh, np.float32)          # [H, C, DH]
    WhT = np.zeros((C, 2 * P), np.float32)   # cols hh*128 + eo*64 + d
    for hh in range(2):
        for eo in range(2):
            WhT[:, hh * P + eo * DH:hh * P + eo * DH + DH] = Wh[2 * hh + eo]
    WkT = np.ascontiguousarray(np.asarray(conv_w, np.float32).transpose(2, 1, 0))
    Ebc = np.zeros((4, C), np.float32)
    for c in range(C):
        Ebc[(c % P) // DH + 2 * (c // P), c] = 1.0
    if conv_b is None:
        cbv = np.zeros((C,), np.float32)
    else:
        cbv = np.asarray(conv_b, np.float32)

    shared = dict(
        Wp=np.asarray(W_sage, np.float16),
        bs=np.asarray(b_sage, np.float32),
        g1=np.asarray(bn1_gamma, np.float32),
        b1=np.asarray(bn1_beta, np.float32),
        WhT=WhT.astype(np.float16),
        WkT=WkT.astype(np.float16),
        cb=cbv,
        g2=np.asarray(bn2_gamma, np.float32),
        b2=np.asarray(bn2_beta, np.float32),
        Ebc=Ebc,
    )
    adjT = adj.T
    in_maps = []
    for core in range(NC):
        b, s = core // 2, core % 2
        shift = s * R - 1  # rotated col i -> global node (i + shift) mod N
        xTr = np.roll(x[b].T, -shift, axis=1)
        adjTr = np.roll(np.roll(adjT, -shift, axis=0), -shift, axis=1)
        emask_v = np.array(
            [0.0 if s == 0 else 1.0, 1.0 if s == 0 else 0.0], np.float32
        )
        m = dict(
            xT=xTr.astype(np.float16),
            adjT=adjTr.astype(np.float16),
            emask=emask_v,
            **shared,
        )
        in_maps.append(m)
    return in_maps


def _assemble(results):
    out = np.empty((B, N, C), np.float32)
    for core in range(NC):
        b, s = core // 2, core % 2
        r = results[core]["out"]  # [P, 2, R]
        out[b, s * R:(s + 1) * R, :] = r.transpose(2, 1, 0).reshape(R, C)
    return out


_CACHE = {}


def kernel(x, adj, W_sage, b_sage, bn1_gamma, bn1_beta, Wh, alpha_gat,
           conv_w, conv_b, bn2_gamma, bn2_beta, alpha_tcn, **_unused):
    ag, at = float(alpha_gat), float(alpha_tcn)
    key = (ag, at)
    if key not in _CACHE:
        _CACHE[key] = build_program(ag, at)
    nc = _CACHE[key]
    in_maps = _prep_inputs(x, adj, W_sage, b_sage, bn1_gamma, bn1_beta, Wh,
                           conv_w, bn2_gamma, bn2_beta, conv_b)
    res = run_bass_kernel_spmd(nc, in_maps, core_ids=list(range(NC)))
    return _assemble(res.results)


if __name__ == "__main__":
    import sys
    sys.path.insert(0, "/root/problem")
    import reference
    inputs = {k: np.asarray(v) for k, v in reference.setup_inputs().items()}
    expected = np.asarray(reference.reference(**inputs))
    actual = kernel(**inputs)
    err = np.abs(actual - expected)
    rel = np.linalg.norm(actual - expected) / np.linalg.norm(expected)
    print("max abs err:", err.max(), "rel:", rel)


# revision 31
# speedup vs baseline: 1.7397x; 1.0177x over previous
"""Trainium2 Bass kernel for nn_GCNWithMultiHeadGATAndTCN_42356967473538.

Sharding: 8 cores = (batch b in 0..3) x (node-half s in 0..1).
Each core computes the FULL batch-b pipeline through BN1 + projections
(redundantly within a pair) so that no activation exchange is needed;
only its own 1024+2 query columns go through attention / TCN / output.

Per-core node axis is ROTATED so that own nodes sit at columns 1..1024
with halo columns 0 and 1025 (edge-masked per core via `emask` input).
This makes the SPMD instruction stream core-uniform; all per-core
differences live in the input data (xT/adjT rotation, emask).

Cross-core communication: only two tiny stats AllGathers ([P,4] f32,
all 8 cores) for the training-mode BatchNorm moments (bn1, bn2); each
core reduces the gathered 8 copies locally.

dtypes: fp16 for x/adj/weights/activations (same 11-bit mantissa as
f32r), f32r for exp/attention values (range), f32 accumulation in PSUM.
"""

import numpy as np

import concourse.bass as bass
import concourse.mybir as mybir
import concourse.tile as tile
from concourse import bacc, dve_ops
from concourse.bass_utils import run_bass_kernel_spmd
from concourse.dve_spec import Spec, Src0, C0, maxx, lower, _has_src1
from concourse.dve_uop import DveOpSpec


def _register_lrelu_op():
    """Custom single-pass DVE leaky-relu: out = max(in0, in0*s0)."""
    if "LRELU_ANT" in dve_ops._SUB_OPCODE_FOR_NAME:
        return dve_ops.CUSTOM_DVE_SPECS and next(
            op for op in dve_ops.OPS if op.name == "LRELU_ANT"
        )
    spec = Spec(
        body=maxx(Src0, Src0 * C0),
        reference=lambda in0, in1, s0, s1, imm2: np.maximum(
            np.nan_to_num(in0, nan=0.0, posinf=np.inf, neginf=-np.inf),
            in0 * s0,
        ).astype(np.float32),
    )
    row = dve_ops._CUSTOM_DVE_ROW_BASE + len(dve_ops.OPS)
    assert row < 0x20
    shas = {}
    for ver in ("v3", "v4"):
        try:
            tmp = DveOpSpec(name="LRELU_ANT", opcode=row, uops=lower(spec, ver=ver),
                            rd1_en=_has_src1(spec))
            shas[ver] = tmp.sha(ver)
        except Exception:
            pass
    op = dve_ops.DveOp("LRELU_ANT", spec, False, shas)
    dve_ops.OPS.append(op)
    dve_ops.CUSTOM_DVE_SPECS["LRELU_ANT"] = spec
    dve_ops._SUB_OPCODE_FOR_NAME["LRELU_ANT"] = row
    return op


LRELU_ANT = _register_lrelu_op()

F32 = mybir.dt.float32
F32R = mybir.dt.float32r
F16 = mybir.dt.float16
AF = mybir.ActivationFunctionType
ALU = mybir.AluOpType
AX = mybir.AxisListType

B, N, FEAT, C, H, DH = 4, 2048, 256, 256, 4, 64
P = 128
R = N // 2            # own nodes per core (1024)
Q = R + 2             # query columns incl. both halos (1026)
NC = 8                # cores
EPS = 1e-5
SLOPE = 0.2
EXP_SHIFT = 64.0      # softmax-invariant shift keeps exp in f32 range
CNT1 = float(2 * B * N)   # bn1 sample count x2 (pairs duplicate batches)
CNT2 = float(B * N)       # bn2 sample count (own node halves, no dup)

ALL8 = [list(range(NC))]

# which lrelu tiles take the ACT (parametric-relu) path; the rest use the
# fused DVE custom op. Balances DVE vs ACT occupancy in phase F.
ACT_T = frozenset({3, 7, 11, 15})


def _bc_ap(ap, parts=P):
    """Broadcast a DRAM AP across `parts` partitions (stride-0 partition dim)."""
    return bass.AP(tensor=ap.tensor, offset=ap.offset, ap=[[0, parts], *ap.ap])


def build_program(alpha_gat: float, alpha_tcn: float, sim_safe: bool = False,
                  **_unused):
    nc = bacc.Bacc(
        "TRN2", target_bir_lowering=False, debug=False, num_devices=NC
    )

    def din(name, shape, dt=F32):
        return nc.dram_tensor(name, shape, dt, kind="ExternalInput").ap()

    xT = din("xT", [FEAT, N], F16)       # x[b].T, node-rotated
    adjT = din("adjT", [N, N], F16)      # adj.T, node-rotated both axes
    Wp = din("Wp", [FEAT, C], F16)
    bs = din("bs", [C])
    g1 = din("g1", [C])
    b1 = din("b1", [C])
    WhT = din("WhT", [C, 2 * P], F16)    # cols = hh*128 + eo*64 + d
    WkT = din("WkT", [3, C, C], F16)     # conv_w[:, :, k].T -> [k, cin, cout]
    cb = din("cb", [C])
    g2 = din("g2", [C])
    b2 = din("b2", [C])
    Ebc = din("Ebc", [4, C], F32R)       # head->channel one-hot (recip bcast)
    emask = din("emask", [2])            # halo-col validity [left, right]

    out = nc.dram_tensor("out", [P, 2, R], F32, kind="ExternalOutput").ap()

    # internal DRAM bounce buffers for the stats collectives
    cc1_in = nc.dram_tensor("cc1_in", [P, 4], F32).ap()
    cc1_out = nc.dram_tensor("cc1_out", [NC, P, 4], F32).ap()
    cc2_in = nc.dram_tensor("cc2_in", [P, 4], F32).ap()
    cc2_out = nc.dram_tensor("cc2_out", [NC, P, 4], F32).ap()

    with tile.TileContext(nc) as tc:
        with (
            tc.tile_pool(name="persist", bufs=1) as ppool,
            tc.tile_pool(name="work", bufs=2) as wpool,
            tc.tile_pool(name="adjp", bufs=4) as adjpool,
            tc.tile_pool(name="expp", bufs=2) as expool,
            tc.tile_pool(name="psum", bufs=1, space="PSUM") as psum,
        ):
            # ---------- constants ----------
            Wp_sb = ppool.tile([P, 2, C], F16, tag="Wp_sb")
            nc.sync.dma_start(Wp_sb[:], Wp.rearrange("(o p) c -> p o c", p=P))
            xT_sb = ppool.tile([P, 2, N], F16, tag="xT_sb")
            xTv = xT.rearrange("(ko p) m -> p ko m", p=P)
            nc.sync.dma_start(xT_sb[:, :, 0:N // 2], xTv[:, :, 0:N // 2])
            nc.sync.dma_start(xT_sb[:, :, N // 2:N], xTv[:, :, N // 2:N])
            # prefetch the first adj chunks before the remaining constants so
            # the (in-order) DMA queue feeds phase B without head-of-line
            # stalls; the rest are issued in the B loop (bufs=4 rotation).
            adj_tiles = []
            for t in range(4):
                at = adjpool.tile([P, N], F16, tag="at", name=f"at{t}")
                nc.sync.dma_start(at[:], adjT[t * P:(t + 1) * P, :])
                adj_tiles.append(at)
            Wh_sb = ppool.tile([P, 2, 2, P], F16, tag="Wh_sb")
            nc.sync.dma_start(
                Wh_sb[:], WhT.rearrange("(o p) c -> p o c", p=P)
            )
            Wk_sb = ppool.tile([P, 2, 3, C], F16, tag="Wk_sb")
            for k in range(3):
                nc.sync.dma_start(
                    Wk_sb[:, :, k, :],
                    WkT[k].rearrange("(o p) c -> p o c", p=P),
                )
            bs_sb = ppool.tile([P, 2], F32, tag="bs_sb")
            nc.sync.dma_start(bs_sb[:], bs.rearrange("(o p) -> p o", p=P))
            g1_sb = ppool.tile([P, 2], F32, tag="g1_sb")
            nc.sync.dma_start(g1_sb[:], g1.rearrange("(o p) -> p o", p=P))
            b1_sb = ppool.tile([P, 2], F32, tag="b1_sb")
            nc.sync.dma_start(b1_sb[:], b1.rearrange("(o p) -> p o", p=P))
            cb_sb = ppool.tile([P, 2], F32, tag="cb_sb")
            nc.sync.dma_start(cb_sb[:], cb.rearrange("(o p) -> p o", p=P))
            g2_sb = ppool.tile([P, 2], F32, tag="g2_sb")
            nc.sync.dma_start(g2_sb[:], g2.rearrange("(o p) -> p o", p=P))
            b2_sb = ppool.tile([P, 2], F32, tag="b2_sb")
            nc.sync.dma_start(b2_sb[:], b2.rearrange("(o p) -> p o", p=P))
            Ebc_sb = ppool.tile([4, C], F32R, tag="Ebc_sb")
            nc.sync.dma_start(Ebc_sb[:], Ebc[:, :])
            em_sb = ppool.tile([P, 2], F32, tag="em_sb")
            nc.sync.dma_start(em_sb[:], _bc_ap(emask[:]))
            cm40 = ppool.tile([P, 1], F32, tag="cm40")
            nc.vector.memset(cm40[:], -EXP_SHIFT)

            # PE warm-up: harmless matmuls on the weight tile so the p-state
            # ramp completes during the input DMAs.
            warm_ps = psum.tile([P, 2, 512], F32, tag="q23", name="warm_ps")
            for _ in range(14):
                nc.tensor.matmul(warm_ps[:, 0, 0:C], Wp_sb[:, 0, 0:P], Wp_sb[:, 0, :],
                                 start=True, stop=True)
            warm_scr = ppool.tile([P, 1], F32, tag="warm_scr")
            nc.vector.reduce_sum(warm_scr[:], warm_ps[:, 0, 0:4], axis=AX.X)

            # ---------- phase A: support = x @ W  [m-part, t, c] ----------
            support = ppool.tile([P, 16, C], F16, tag="support")
            for t in range(16):
                psb = psum.tile([P, 2, 512], F32, tag=("q01", "q23")[t % 2],
                                name="ps_sup")
                ps = psb[:, 0, 0:C]
                for ko in range(2):
                    nc.tensor.matmul(
                        ps, xT_sb[:, ko, t * P:(t + 1) * P], Wp_sb[:, ko, :],
                        start=(ko == 0), stop=(ko == 1),
                    )
                if t % 2 == 0:
                    nc.scalar.activation(out=support[:, t, :], in_=ps,
                                         func=AF.Copy)
                else:
                    nc.vector.tensor_copy(out=support[:, t, :], in_=ps)

            # ---------- phase B: hT = relu(support^T @ adjT + b)  [c, n] ----
            hT = ppool.tile([P, 2, N], F16, tag="hT")
            st1 = ppool.tile([P, 2, 4], F32, tag="st1")
            psB = [
                psum.tile([P, 2, 512], F32, tag=tg, name=f"psB_{tg}")
                for tg in ("q01", "q23", "agg01", "x01")
            ]
            ps_b = [
                [psB[(o * 4 + w4) // 2][:, (o * 4 + w4) % 2, :]
                 for w4 in range(4)]
                for o in range(2)
            ]
            st1sq = ppool.tile([P, 2, 4], F32, tag="st1sq")
            sqscr = ppool.tile([P, N], F16, tag="sqscr")
            for t in range(16):
                if t < 4:
                    at = adj_tiles[t]
                else:
                    at = adjpool.tile([P, N], F16, tag="at")
                    nc.sync.dma_start(at[:], adjT[t * P:(t + 1) * P, :])
                for o in range(2):
                    for w4 in range(4):
                        nc.tensor.matmul(
                            ps_b[o][w4],
                            support[:, t, o * P:(o + 1) * P],
                            at[:, w4 * 512:(w4 + 1) * 512],
                            start=(t == 0), stop=(t == 15),
                        )
            for o in range(2):
                for w4 in range(4):
                    sl = slice(w4 * 512, (w4 + 1) * 512)
                    nc.scalar.activation(
                        out=hT[:, o, sl],
                        in_=ps_b[o][w4],
                        func=AF.Relu,
                        bias=bs_sb[:, o:o + 1],
                        accum_out=st1[:, o, w4:w4 + 1],
                    )
                    # sum of squares on DVE, in parallel with the ACT pass
                    nc.vector.affine_mul_reduce(
                        out=sqscr[:, sl], accum_out=st1sq[:, o, w4:w4 + 1],
                        in0=hT[:, o, sl], in1=hT[:, o, sl],
                        scale=1.0, bias=0.0,
                    )

            # ---------- phase C: BN1 stats + allgather + apply ----------
            stp1 = ppool.tile([P, 4], F32, tag="stp1")
            for o in range(2):
                nc.vector.reduce_sum(stp1[:, o:o + 1], st1[:, o, :], axis=AX.X)
                nc.vector.reduce_sum(
                    stp1[:, 2 + o:3 + o], st1sq[:, o, :], axis=AX.X
                )
            nc.sync.dma_start(cc1_in[:, :], stp1[:])
            nc.gpsimd.collective_compute(
                "AllGather", ALU.bypass, replica_groups=ALL8,
                ins=[cc1_in.opt()], outs=[cc1_out.opt()],
            )
            warm2 = psum.tile([P, 2, 512], F32, tag="q23", name="warm2")
            for _ in range(60):
                nc.tensor.matmul(warm2[:, 0, 0:C], Wp_sb[:, 0, 0:P], Wp_sb[:, 0, :],
                                 start=True, stop=True)
            warm2_scr = ppool.tile([P, 1], F32, tag="warm_scr", name="w2scr")
            nc.vector.reduce_sum(warm2_scr[:], warm2[:, 0, 0:4], axis=AX.X)
            stg1g = ppool.tile([P, NC, 4], F32, tag="stg1g")
            nc.sync.dma_start(
                stg1g[:],
                bass.AP(tensor=cc1_out.tensor, offset=cc1_out.offset,
                        ap=[[4, P], [P * 4, NC], [1, 4]]),
            )
            stg1 = ppool.tile([P, 4], F32, tag="stg1")
            for c4 in range(4):
                nc.vector.reduce_sum(
                    stg1[:, c4:c4 + 1], stg1g[:, :, c4], axis=AX.X
                )

            def bn_affine(stg, gam, bet, tagp, cnt, fold=1.0):
                """A, C with y = relu(x*A + C) == relu(fold*bn(x))."""
                mean = ppool.tile([P, 2], F32, tag=f"{tagp}_mean")
                nc.vector.tensor_scalar_mul(mean[:], stg[:, 0:2], 1.0 / cnt)
                ex2 = ppool.tile([P, 2], F32, tag=f"{tagp}_ex2")
                nc.vector.tensor_scalar_mul(ex2[:], stg[:, 2:4], 1.0 / cnt)
                var = ppool.tile([P, 2], F32, tag=f"{tagp}_var")
                nc.vector.tensor_tensor(var[:], mean[:], mean[:], ALU.mult)
                nc.vector.tensor_tensor(var[:], ex2[:], var[:], ALU.subtract)
                rstd = ppool.tile([P, 2], F32, tag=f"{tagp}_rstd")
                nc.vector.tensor_scalar_add(var[:], var[:], EPS)
                nc.scalar.activation(rstd[:], var[:], AF.Ln)
                nc.scalar.activation(rstd[:], rstd[:], AF.Exp, scale=-0.5)
                A = ppool.tile([P, 2], F32, tag=f"{tagp}_A")
                nc.vector.tensor_tensor(A[:], gam[:], rstd[:], ALU.mult)
                Cc = ppool.tile([P, 2], F32, tag=f"{tagp}_C")
                nc.vector.tensor_tensor(Cc[:], mean[:], A[:], ALU.mult)
                nc.vector.tensor_tensor(Cc[:], bet[:], Cc[:], ALU.subtract)
                if fold != 1.0:
                    nc.vector.tensor_scalar_mul(A[:], A[:], fold)
                    nc.vector.tensor_scalar_mul(Cc[:], Cc[:], fold)
                return A, Cc

            A1, C1 = bn_affine(stg1, g1_sb, b1_sb, "bn1", CNT1)
            hbn = ppool.tile([P, 2, N], F16, tag="hbn")
            for w4 in range(4):
                for o in range(2):
                    sl = slice(w4 * 512, (w4 + 1) * 512)
                    nc.scalar.activation(
                        out=hbn[:, o, sl], in_=hT[:, o, sl], func=AF.Relu,
                        scale=A1[:, o:o + 1], bias=C1[:, o:o + 1],
                    )

            # ---------- phase E: hp projections ----------
            # hpTf: [d-major (eo*64+d), hh, n]  (fp16, e-matmul operands)
            hpTf = ppool.tile([P, 2, N], F16, tag="hpTf")
            for hh in range(2):
                for w4 in range(4):
                    psb = psum.tile([P, 2, 512], F32,
                                    tag=("q01", "q23")[(hh * 4 + w4) % 2],
                                    name="ps_hpf")
                    ps = psb[:, 0, :]
                    for o in range(2):
                        nc.tensor.matmul(
                            ps,
                            Wh_sb[:, o, hh, :],
                            hbn[:, o, w4 * 512:(w4 + 1) * 512],
                            start=(o == 0), stop=(o == 1),
                        )
                    nc.vector.tensor_copy(
                        out=hpTf[:, hh, w4 * 512:(w4 + 1) * 512], in_=ps
                    )
            # hpA: node-major [n-part, t, hh, 130] f32r with ones cols 64/129
            hpA = ppool.tile([P, 16, 2, 130], F32R, tag="hpA")
            ones1 = ppool.tile([P, 1], F32, tag="ones1")
            nc.vector.memset(ones1[:], 1.0)
            ones_src = bass.AP(
                tensor=ones1.tensor, offset=ones1.offset,
                ap=[ones1.ap[0], [0, 16], [0, 2]],
            )
            for col in (64, 129):
                onesv = bass.AP(
                    tensor=hpA.tensor, offset=hpA.offset + col,
                    ap=[hpA.ap[0], [260, 16], [130, 2]],
                )  # [p, t, hh] at fixed col
                nc.vector.tensor_copy(out=onesv, in_=ones_src)
            for t in range(16):
                psb = psum.tile([P, 2, 512], F32, tag=("agg01", "x01")[t % 2],
                                name="ps_hpa")
                ps = psb[:, 0, 0:C]
                for o in range(2):
                    nc.tensor.matmul(
                        ps,
                        hbn[:, o, t * P:(t + 1) * P],
                        Wh_sb[:, o, :, :].rearrange("p hh x -> p (hh x)"),
                        start=(o == 0), stop=(o == 1),
                    )
                # psum cols (hh, eo, d) -> hpA[:, t, hh, eo*65 + d]
                psv = ps.rearrange("p (hh eo d) -> p hh eo d", hh=2, eo=2)
                dst = bass.AP(
                    tensor=hpA.tensor, offset=hpA.offset + t * 260,
                    ap=[hpA.ap[0], [130, 2], [65, 2], [1, 64]],
                )
                if t % 2 == 0:
                    nc.scalar.activation(out=dst, in_=psv[:], func=AF.Copy)
                else:
                    nc.vector.tensor_copy(out=dst, in_=psv[:])

            # ---------- phase F: attention fused with g/conv/bn2-stats ------
            aggT = ppool.tile([P, 2, Q], F32, tag="aggT")
            den4 = ppool.tile([4, Q], F32, tag="den4")
            rec4f = ppool.tile([4, Q], F32, tag="rec4f")
            rscr = ppool.tile([4, Q], F32, tag="rscr")
            rec4 = ppool.tile([4, Q], F32R, tag="rec4")
            g = ppool.tile([P, 2, Q], F16, tag="g")
            tconv = ppool.tile([P, 2, R], F16, tag="tconv")
            st2 = ppool.tile([P, 2, 3], F32, tag="st2")
            st2sq = ppool.tile([P, 2, 3], F32, tag="st2sq")
            emv = bass.AP(
                tensor=em_sb.tensor, offset=em_sb.offset,
                ap=[em_sb.ap[0], [0, 2], [1, 1]],
            )  # [p, o(bc), 1] at left-mask col; offset +1 for right

            def attn_block(hh, qb):
                """e/lrelu/exp/agg for head pair hh, query cols [qb, qb+512)."""
                agg2 = psum.tile([P, 2, 512], F32, tag="agg01", name="agg2")
                for g4 in range(4):
                    # el is eo-major so the in-place exp runs on contiguous
                    # [P, 4, 512] slabs (the hardware-validated pattern)
                    el = expool.tile([P, 2, 4, 512], F32R, tag="el")
                    for tt in range(4):
                        t = g4 * 4 + tt
                        eg2 = psum.tile([P, 2, 512], F32,
                                        tag=("q01", "q23")[t % 2], name="eg2")
                        nc.tensor.matmul(
                            eg2[:, 0, :],
                            hpTf[0:64, hh, t * P:(t + 1) * P],
                            hpTf[0:64, hh, qb:qb + 512],
                            start=True, stop=True,
                        )
                        nc.tensor.matmul(
                            eg2[:, 1, :],
                            hpTf[64:128, hh, t * P:(t + 1) * P],
                            hpTf[64:128, hh, qb:qb + 512],
                            start=True, stop=True,
                        )
                        # leaky-relu evacuation psum -> sbuf f32r; some tiles
                        # take the ACT parametric-relu path to balance engines
                        if t in ACT_T:
                            for eo in range(2):
                                nc.scalar.activation(
                                    out=el[:, eo, tt, :], in_=eg2[:, eo, :],
                                    func=AF.Prelu, alpha=SLOPE,
                                )
                        else:
                            nc.vector._custom_dve(
                                LRELU_ANT, out=el[:, :, tt, :], in0=eg2[:],
                                s0=SLOPE,
                            )
                    for eo in range(2):
                        nc.scalar.activation(out=el[:, eo, :, :],
                                             in_=el[:, eo, :, :],
                                             func=AF.Exp, bias=cm40[:])
                        for tt in range(4):
                            t = g4 * 4 + tt
                            nc.tensor.matmul(
                                agg2[0:65, eo, :],
                                hpA[:, t, hh, eo * 65:(eo + 1) * 65],
                                el[:, eo, tt, :],
                                start=(t == 0), stop=(t == 15),
                            )
                # drain agg (psum -> sbuf stage -> aggT/den rows via DMA)
                stage2 = wpool.tile([65, 2, 512], F32, tag="stage",
                                    name="stage2")
                nc.vector.tensor_copy(out=stage2[:], in_=agg2[0:65, :, :])
                nc.sync.dma_start(aggT[0:64, hh, qb:qb + 512], stage2[0:64, 0, :])
                nc.sync.dma_start(
                    den4[2 * hh:2 * hh + 1, qb:qb + 512], stage2[64:65, 0, :]
                )
                nc.sync.dma_start(aggT[64:128, hh, qb:qb + 512], stage2[0:64, 1, :])
                nc.sync.dma_start(
                    den4[2 * hh + 1:2 * hh + 2, qb:qb + 512], stage2[64:65, 1, :]
                )

            def g_chunk(qb, ws):
                """recip + g assembly for query cols [qb, qb+ws)."""
                nc.vector.reciprocal_approx_accurate(
                    out=rec4f[:, qb:qb + ws], in_=den4[:, qb:qb + ws],
                    scratch=rscr[:, qb:qb + ws],
                )
                nc.vector.tensor_scalar_mul(
                    rec4[:, qb:qb + ws], rec4f[:, qb:qb + ws], float(alpha_gat)
                )
                for o in range(2):
                    bcb = psum.tile([P, 2, 512], F32, tag="x01", name="bc")
                    bc = bcb[:, 0, 0:ws]
                    nc.tensor.matmul(
                        bc,
                        Ebc_sb[:, o * P:(o + 1) * P],
                        rec4[:, qb:qb + ws],
                        start=True, stop=True,
                    )
                    gsl = g[:, o, qb:qb + ws]
                    nc.vector.tensor_tensor(
                        gsl, aggT[:, o, qb:qb + ws], bc, ALU.mult
                    )
                    nc.vector._custom_dve(
                        dve_ops.AFFINE_THEN_ADD,
                        out=gsl,
                        in0=hbn[:, o, qb:qb + ws],
                        in1=gsl,
                        s0=float(1.0 - alpha_gat),
                        s1=0.0,
                    )

            def mask_edge(col, mi):
                """halo-edge validity mask on g col (mi: 0=left, 1=right)."""
                gsl = bass.AP(
                    tensor=g.tensor, offset=g.offset + col,
                    ap=[g.ap[0], [Q, 2], [1, 1]],
                )
                mv = bass.AP(
                    tensor=em_sb.tensor, offset=em_sb.offset + mi,
                    ap=[em_sb.ap[0], [0, 2], [1, 1]],
                )
                nc.vector.tensor_tensor(gsl, gsl, mv, ALU.mult)

            CONV_CHUNKS = ((1, 511), (511, 1021), (1021, 1025))

            def conv_chunk(ci):
                """conv output cols [a, b) -> tconv cols [a-1, b-1) + stats."""
                a, b_ = CONV_CHUNKS[ci]
                width = b_ - a
                cvb = psum.tile([P, 2, 512], F32, tag="x01", name="ps_cv")
                for oo in range(2):
                    first = True
                    for oi in range(2):
                        for k in range(3):
                            nc.tensor.matmul(
                                cvb[:, oo, 0:width],
                                Wk_sb[:, oi, k, oo * P:(oo + 1) * P],
                                g[:, oi, a - 1 + k:b_ - 1 + k],
                                start=first, stop=(oi == 1 and k == 2),
                            )
                            first = False
                    # conv_b omitted: bn2 (training mode) follows directly, so
                    # a per-channel constant shift cancels exactly.
                    tsl = tconv[:, oo, a - 1:b_ - 1]
                    nc.scalar.activation(
                        out=tsl, in_=cvb[:, oo, 0:width], func=AF.Copy,
                        accum_out=st2[:, oo, ci:ci + 1],
                    )
                    nc.vector.affine_mul_reduce(
                        out=sqscr[:, 0:width],
                        accum_out=st2sq[:, oo, ci:ci + 1],
                        in0=tsl, in1=tsl, scale=1.0, bias=0.0,
                    )

            for w in range(2):
                qb = w * 512
                for hh in range(2):
                    attn_block(hh, qb)
                g_chunk(qb, 512)
                if w == 0:
                    mask_edge(0, 0)
                conv_chunk(w)

            # ----- halo queries (cols 1024, 1025) -----
            # eh_ps/elH cols: hh*64 + eo*32 + t*2 + q
            ehb = psum.tile([P, 2, 512], F32, tag="q01", name="eh_ps")
            for hh in range(2):
                for eo in range(2):
                    sl = slice(eo * 64, (eo + 1) * 64)
                    for t in range(16):
                        cbase = hh * 64 + eo * 32 + t * 2
                        nc.tensor.matmul(
                            ehb[:, 0, cbase:cbase + 2],
                            hpTf[sl, hh, t * P:(t + 1) * P],
                            hpTf[sl, hh, R:R + 2],
                            start=True, stop=True,
                        )
            elH = expool.tile([P, 128], F32R, tag="elH")
            nc.vector._custom_dve(LRELU_ANT, out=elH[:], in0=ehb[:, 0, 0:128], s0=SLOPE)
            nc.scalar.activation(out=elH[:], in_=elH[:], func=AF.Exp,
                                 bias=cm40[:])
            aggHb = psum.tile([P, 2, 512], F32, tag="agg01", name="aggH")
            for hh in range(2):
                for eo in range(2):
                    for t in range(16):
                        cbase = hh * 64 + eo * 32 + t * 2
                        hbase = (2 * hh + eo) * 2
                        nc.tensor.matmul(
                            aggHb[0:65, 0, hbase:hbase + 2],
                            hpA[:, t, hh, eo * 65:(eo + 1) * 65],
                            elH[:, cbase:cbase + 2],
                            start=(t == 0), stop=(t == 15),
                        )
            stageH = wpool.tile([65, 8], F32, tag="stageH")
            nc.vector.tensor_copy(out=stageH[:], in_=aggHb[0:65, 0, 0:8])
            nc.sync.dma_start(den4[0:4, R:R + 2], stageH[64:65, :])
            for hh in range(2):
                nc.vector.tensor_copy(
                    out=aggT[0:64, hh, R:R + 2],
                    in_=stageH[0:64, 4 * hh:4 * hh + 2],
                )
                nc.sync.dma_start(
                    aggT[64:128, hh, R:R + 2],
                    stageH[0:64, 4 * hh + 2:4 * hh + 4],
                )

            # ----- halo g + last conv chunk + BN2 stats finalize -----
            g_chunk(R, 2)
            mask_edge(Q - 1, 1)
            conv_chunk(2)

            # ---------- phase I: BN2 + residual + output ----------
            stp2 = ppool.tile([P, 4], F32, tag="stp2")
            for o in range(2):
                nc.vector.reduce_sum(stp2[:, o:o + 1], st2[:, o, :], axis=AX.X)
                nc.vector.reduce_sum(
                    stp2[:, 2 + o:3 + o], st2sq[:, o, :], axis=AX.X
                )
            nc.sync.dma_start(cc2_in[:, :], stp2[:])
            nc.gpsimd.collective_compute(
                "AllGather", ALU.bypass, replica_groups=ALL8,
                ins=[cc2_in.opt()], outs=[cc2_out.opt()],
            )
            stg2g = ppool.tile([P, NC, 4], F32, tag="stg2g")
            nc.sync.dma_start(
                stg2g[:],
                bass.AP(tensor=cc2_out.tensor, offset=cc2_out.offset,
                        ap=[[4, P], [P * 4, NC], [1, 4]]),
            )
            stg2 = ppool.tile([P, 4], F32, tag="stg2")
            for c4 in range(4):
                nc.vector.reduce_sum(
                    stg2[:, c4:c4 + 1], stg2g[:, :, c4], axis=AX.X
                )
            fold = alpha_tcn if alpha_tcn > 0 else 1.0
            A2, C2 = bn_affine(stg2, g2_sb, b2_sb, "bn2", CNT2, fold=fold)

            fin = ppool.tile([P, 2, R], F32, tag="fin")
            HR = R // 2
            for o in range(2):
                for q2 in range(2):
                    sl = slice(q2 * HR, (q2 + 1) * HR)
                    nc.scalar.activation(
                        out=fin[:, o, sl], in_=tconv[:, o, sl], func=AF.Relu,
                        scale=A2[:, o:o + 1], bias=C2[:, o:o + 1],
                    )
                    if fold != alpha_tcn:  # alpha_tcn <= 0: scale separately
                        nc.vector.tensor_scalar_mul(
                            fin[:, o, sl], fin[:, o, sl], float(alpha_tcn)
                        )
                    nc.vector._custom_dve(
                        dve_ops.AFFINE_THEN_ADD,
                        out=fin[:, o, sl],
                        in0=g[:, o, 1 + q2 * HR:1 + (q2 + 1) * HR],
                        in1=fin[:, o, sl],
                        s0=float(1.0 - alpha_tcn),
                        s1=0.0,
                    )
                    nc.sync.dma_start(out[:, o, sl], fin[:, o, sl])

    nc.compile()
    return nc


def _prep_inputs(x, adj, W_sage, b_sage, bn1_gamma, bn1_beta, Wh,
                 conv_w, bn2_gamma, bn2_beta, conv_b=None):
    """Build the 8 per-core input maps (host-side numpy)."""
    x = np.asarray(x, np.float32)
    adj = np.asarray(adj, np.float32)
    Wh = np.asarray(Wh, np.float32)          # [H, C, DH]
    WhT = np.zeros((C, 2 * P), np.float32)   # cols hh*128 + eo*64 + d
    for hh in range(2):
        for eo in range(2):
            WhT[:, hh * P + eo * DH:hh * P + eo * DH + DH] = Wh[2 * hh + eo]
    WkT = np.ascontiguousarray(np.asarray(conv_w, np.float32).transpose(2, 1, 0))
    Ebc = np.zeros((4, C), np.float32)
    for c in range(C):
        Ebc[(c % P) // DH + 2 * (c // P), c] = 1.0
    if conv_b is None:
        cbv = np.zeros((C,), np.float32)
    else:
        cbv = np.asarray(conv_b, np.float32)

    shared = dict(
        Wp=np.asarray(W_sage, np.float16),
        bs=np.asarray(b_sage, np.float32),
        g1=np.asarray(bn1_gamma, np.float32),
        b1=np.asarray(bn1_beta, np.float32),
        WhT=WhT.astype(np.float16),
        WkT=WkT.astype(np.float16),
        cb=cbv,
        g2=np.asarray(bn2_gamma, np.float32),
        b2=np.asarray(bn2_beta, np.float32),
        Ebc=Ebc,
    )
    adjT = adj.T
    in_maps = []
    for core in range(NC):
        b, s = core // 2, core % 2
        shift = s * R - 1  # rotated col i -> global node (i + shift) mod N
        xTr = np.roll(x[b].T, -shift, axis=1)
        adjTr = np.roll(np.roll(adjT, -shift, axis=0), -shift, axis=1)
        emask_v = np.array(
            [0.0 if s == 0 else 1.0, 1.0 if s == 0 else 0.0], np.float32
        )
        m = dict(
            xT=xTr.astype(np.float16),
            adjT=adjTr.astype(np.float16),
            emask=emask_v,
            **shared,
        )
        in_maps.append(m)
    return in_maps


def _assemble(results):
    out = np.empty((B, N, C), np.float32)
    for core in range(NC):
        b, s = core // 2, core % 2
        r = results[core]["out"]  # [P, 2, R]
        out[b, s * R:(s + 1) * R, :] = r.transpose(2, 1, 0).reshape(R, C)
    return out


_CACHE = {}


def kernel(x, adj, W_sage, b_sage, bn1_gamma, bn1_beta, Wh, alpha_gat,
           conv_w, conv_b, bn2_gamma, bn2_beta, alpha_tcn, **_unused):
    ag, at = float(alpha_gat), float(alpha_tcn)
    key = (ag, at)
    if key not in _CACHE:
        _CACHE[key] = build_program(ag, at)
    nc = _CACHE[key]
    in_maps = _prep_inputs(x, adj, W_sage, b_sage, bn1_gamma, bn1_beta, Wh,
                           conv_w, bn2_gamma, bn2_beta, conv_b)
    res = run_bass_kernel_spmd(nc, in_maps, core_ids=list(range(NC)))
    return _assemble(res.results)


if __name__ == "__main__":
    import sys
    sys.path.insert(0, "/root/problem")
    import reference
    inputs = {k: np.asarray(v) for k, v in reference.setup_inputs().items()}
    expected = np.asarray(reference.reference(**inputs))
    actual = kernel(**inputs)
    err = np.abs(actual - expected)
    rel = np.linalg.norm(actual - expected) / np.linalg.norm(expected)
    print("max abs err:", err.max(), "rel:", rel)


# revision 36
# speedup vs baseline: 1.7544x; 1.0084x over previous
"""Trainium2 Bass kernel for nn_GCNWithMultiHeadGATAndTCN_42356967473538.

Sharding: 8 cores = (batch b in 0..3) x (node-half s in 0..1).
Each core computes the FULL batch-b pipeline through BN1 + projections
(redundantly within a pair) so that no activation exchange is needed;
only its own 1024+2 query columns go through attention / TCN / output.

Per-core node axis is ROTATED so that own nodes sit at columns 1..1024
with halo columns 0 and 1025 (edge-masked per core via `emask` input).
This makes the SPMD instruction stream core-uniform; all per-core
differences live in the input data (xT/adjT rotation, emask).

Cross-core communication: only two tiny stats AllGathers ([P,4] f32,
all 8 cores) for the training-mode BatchNorm moments (bn1, bn2); each
core reduces the gathered 8 copies locally.

dtypes: fp16 for x/adj/weights/activations (same 11-bit mantissa as
f32r), f32r for exp/attention values (range), f32 accumulation in PSUM.
"""

import numpy as np

import concourse.bass as bass
import concourse.mybir as mybir
import concourse.tile as tile
from concourse import bacc, dve_ops
from concourse.bass_utils import run_bass_kernel_spmd
from concourse.dve_spec import Spec, Src0, C0, maxx, lower, _has_src1
from concourse.dve_uop import DveOpSpec


def _register_lrelu_op():
    """Custom single-pass DVE leaky-relu: out = max(in0, in0*s0)."""
    if "LRELU_ANT" in dve_ops._SUB_OPCODE_FOR_NAME:
        return dve_ops.CUSTOM_DVE_SPECS and next(
            op for op in dve_ops.OPS if op.name == "LRELU_ANT"
        )
    spec = Spec(
        body=maxx(Src0, Src0 * C0),
        reference=lambda in0, in1, s0, s1, imm2: np.maximum(
            np.nan_to_num(in0, nan=0.0, posinf=np.inf, neginf=-np.inf),
            in0 * s0,
        ).astype(np.float32),
    )
    row = dve_ops._CUSTOM_DVE_ROW_BASE + len(dve_ops.OPS)
    assert row < 0x20
    shas = {}
    for ver in ("v3", "v4"):
        try:
            tmp = DveOpSpec(name="LRELU_ANT", opcode=row, uops=lower(spec, ver=ver),
                            rd1_en=_has_src1(spec))
            shas[ver] = tmp.sha(ver)
        except Exception:
            pass
    op = dve_ops.DveOp("LRELU_ANT", spec, False, shas)
    dve_ops.OPS.append(op)
    dve_ops.CUSTOM_DVE_SPECS["LRELU_ANT"] = spec
    dve_ops._SUB_OPCODE_FOR_NAME["LRELU_ANT"] = row
    return op


LRELU_ANT = _register_lrelu_op()

F32 = mybir.dt.float32
F32R = mybir.dt.float32r
F16 = mybir.dt.float16
AF = mybir.ActivationFunctionType
ALU = mybir.AluOpType
AX = mybir.AxisListType

B, N, FEAT, C, H, DH = 4, 2048, 256, 256, 4, 64
P = 128
R = N // 2            # own nodes per core (1024)
Q = R + 2             # query columns incl. both halos (1026)
NC = 8                # cores
EPS = 1e-5
SLOPE = 0.2
EXP_SHIFT = 64.0      # softmax-invariant shift keeps exp in f32 range
CNT1 = float(2 * B * N)   # bn1 sample count x2 (pairs duplicate batches)
CNT2 = float(B * N)       # bn2 sample count (own node halves, no dup)

ALL8 = [list(range(NC))]

# which lrelu tiles take the ACT (parametric-relu) path; the rest use the
# fused DVE custom op. Balances DVE vs ACT occupancy in phase F.
ACT_T = frozenset({3, 7, 11, 15})


def _bc_ap(ap, parts=P):
    """Broadcast a DRAM AP across `parts` partitions (stride-0 partition dim)."""
    return bass.AP(tensor=ap.tensor, offset=ap.offset, ap=[[0, parts], *ap.ap])


def build_program(alpha_gat: float, alpha_tcn: float, sim_safe: bool = False,
                  **_unused):
    nc = bacc.Bacc(
        "TRN2", target_bir_lowering=False, debug=False, num_devices=NC
    )

    def din(name, shape, dt=F32):
        return nc.dram_tensor(name, shape, dt, kind="ExternalInput").ap()

    xT = din("xT", [FEAT, N], F16)       # x[b].T, node-rotated
    adjT = din("adjT", [N, N], F16)      # adj.T, node-rotated both axes
    Wp = din("Wp", [FEAT, C], F16)
    bs = din("bs", [C])
    g1 = din("g1", [C])
    b1 = din("b1", [C])
    WhT = din("WhT", [C, 2 * P], F16)    # cols = hh*128 + eo*64 + d
    WkT = din("WkT", [3, C, C], F16)     # conv_w[:, :, k].T -> [k, cin, cout]
    cb = din("cb", [C])
    g2 = din("g2", [C])
    b2 = din("b2", [C])
    Ebc = din("Ebc", [4, C], F32R)       # head->channel one-hot (recip bcast)
    emask = din("emask", [2])            # halo-col validity [left, right]

    out = nc.dram_tensor("out", [P, 2, R], F32, kind="ExternalOutput").ap()

    # internal DRAM bounce buffers for the stats collectives
    cc1_in = nc.dram_tensor("cc1_in", [P, 4], F32).ap()
    cc1_out = nc.dram_tensor("cc1_out", [NC, P, 4], F32).ap()
    cc2_in = nc.dram_tensor("cc2_in", [P, 4], F32).ap()
    cc2_out = nc.dram_tensor("cc2_out", [NC, P, 4], F32).ap()

    with tile.TileContext(nc) as tc:
        with (
            tc.tile_pool(name="persist", bufs=1) as ppool,
            tc.tile_pool(name="work", bufs=2) as wpool,
            tc.tile_pool(name="adjp", bufs=4) as adjpool,
            tc.tile_pool(name="expp", bufs=2) as expool,
            tc.tile_pool(name="psum", bufs=1, space="PSUM") as psum,
        ):
            # ---------- constants ----------
            Wp_sb = ppool.tile([P, 2, C], F16, tag="Wp_sb")
            nc.sync.dma_start(Wp_sb[:], Wp.rearrange("(o p) c -> p o c", p=P))
            xT_sb = ppool.tile([P, 2, N], F16, tag="xT_sb")
            xTv = xT.rearrange("(ko p) m -> p ko m", p=P)
            nc.sync.dma_start(xT_sb[:, :, 0:N // 2], xTv[:, :, 0:N // 2])
            nc.sync.dma_start(xT_sb[:, :, N // 2:N], xTv[:, :, N // 2:N])
            # prefetch the first adj chunks before the remaining constants so
            # the (in-order) DMA queue feeds phase B without head-of-line
            # stalls; the rest are issued in the B loop (bufs=4 rotation).
            adj_tiles = []
            for t in range(4):
                at = adjpool.tile([P, N], F16, tag="at", name=f"at{t}")
                nc.sync.dma_start(at[:], adjT[t * P:(t + 1) * P, :])
                adj_tiles.append(at)
            Wh_sb = ppool.tile([P, 2, 2, P], F16, tag="Wh_sb")
            nc.sync.dma_start(
                Wh_sb[:], WhT.rearrange("(o p) c -> p o c", p=P)
            )
            Wk_sb = ppool.tile([P, 2, 3, C], F16, tag="Wk_sb")
            for k in range(3):
                nc.sync.dma_start(
                    Wk_sb[:, :, k, :],
                    WkT[k].rearrange("(o p) c -> p o c", p=P),
                )
            bs_sb = ppool.tile([P, 2], F32, tag="bs_sb")
            nc.sync.dma_start(bs_sb[:], bs.rearrange("(o p) -> p o", p=P))
            g1_sb = ppool.tile([P, 2], F32, tag="g1_sb")
            nc.sync.dma_start(g1_sb[:], g1.rearrange("(o p) -> p o", p=P))
            b1_sb = ppool.tile([P, 2], F32, tag="b1_sb")
            nc.sync.dma_start(b1_sb[:], b1.rearrange("(o p) -> p o", p=P))
            cb_sb = ppool.tile([P, 2], F32, tag="cb_sb")
            nc.sync.dma_start(cb_sb[:], cb.rearrange("(o p) -> p o", p=P))
            g2_sb = ppool.tile([P, 2], F32, tag="g2_sb")
            nc.sync.dma_start(g2_sb[:], g2.rearrange("(o p) -> p o", p=P))
            b2_sb = ppool.tile([P, 2], F32, tag="b2_sb")
            nc.sync.dma_start(b2_sb[:], b2.rearrange("(o p) -> p o", p=P))
            Ebc_sb = ppool.tile([4, C], F32R, tag="Ebc_sb")
            nc.sync.dma_start(Ebc_sb[:], Ebc[:, :])
            em_sb = ppool.tile([P, 2], F32, tag="em_sb")
            nc.sync.dma_start(em_sb[:], _bc_ap(emask[:]))
            cm40 = ppool.tile([P, 1], F32, tag="cm40")
            nc.vector.memset(cm40[:], -EXP_SHIFT)

            # PE warm-up: harmless matmuls on the weight tile so the p-state
            # ramp completes during the input DMAs.
            warm_ps = psum.tile([P, 2, 512], F32, tag="q23", name="warm_ps")
            for _ in range(14):
                nc.tensor.matmul(warm_ps[:, 0, 0:C], Wp_sb[:, 0, 0:P], Wp_sb[:, 0, :],
                                 start=True, stop=True)
            warm_scr = ppool.tile([P, 1], F32, tag="warm_scr")
            nc.vector.reduce_sum(warm_scr[:], warm_ps[:, 0, 0:4], axis=AX.X)

            # ---------- phase A: support = x @ W  [m-part, t, c] ----------
            support = ppool.tile([P, 16, C], F16, tag="support")
            for t in range(16):
                psb = psum.tile([P, 2, 512], F32, tag=("q01", "q23")[t % 2],
                                name="ps_sup")
                ps = psb[:, 0, 0:C]
                for ko in range(2):
                    nc.tensor.matmul(
                        ps, xT_sb[:, ko, t * P:(t + 1) * P], Wp_sb[:, ko, :],
                        start=(ko == 0), stop=(ko == 1),
                    )
                if t % 2 == 0:
                    nc.scalar.activation(out=support[:, t, :], in_=ps,
                                         func=AF.Copy)
                else:
                    nc.vector.tensor_copy(out=support[:, t, :], in_=ps)

            # ---------- phase B: hT = relu(support^T @ adjT + b)  [c, n] ----
            hT = ppool.tile([P, 2, N], F16, tag="hT")
            st1 = ppool.tile([P, 2, 4], F32, tag="st1")
            psB = [
                psum.tile([P, 2, 512], F32, tag=tg, name=f"psB_{tg}")
                for tg in ("q01", "q23", "agg01", "x01")
            ]
            ps_b = [
                [psB[(o * 4 + w4) // 2][:, (o * 4 + w4) % 2, :]
                 for w4 in range(4)]
                for o in range(2)
            ]
            st1sq = ppool.tile([P, 2, 4], F32, tag="st1sq")
            sqscr = ppool.tile([P, N], F16, tag="sqscr")
            for t in range(16):
                if t < 4:
                    at = adj_tiles[t]
                else:
                    at = adjpool.tile([P, N], F16, tag="at")
                    nc.sync.dma_start(at[:], adjT[t * P:(t + 1) * P, :])
                for o in range(2):
                    for w4 in range(4):
                        nc.tensor.matmul(
                            ps_b[o][w4],
                            support[:, t, o * P:(o + 1) * P],
                            at[:, w4 * 512:(w4 + 1) * 512],
                            start=(t == 0), stop=(t == 15),
                        )
            for o in range(2):
                for w4 in range(4):
                    sl = slice(w4 * 512, (w4 + 1) * 512)
                    nc.scalar.activation(
                        out=hT[:, o, sl],
                        in_=ps_b[o][w4],
                        func=AF.Relu,
                        bias=bs_sb[:, o:o + 1],
                        accum_out=st1[:, o, w4:w4 + 1],
                    )
                    # sum of squares on DVE, in parallel with the ACT pass
                    nc.vector.affine_mul_reduce(
                        out=sqscr[:, sl], accum_out=st1sq[:, o, w4:w4 + 1],
                        in0=hT[:, o, sl], in1=hT[:, o, sl],
                        scale=1.0, bias=0.0,
                    )

            # ---------- phase C: BN1 stats + allgather + apply ----------
            stp1 = ppool.tile([P, 4], F32, tag="stp1")
            for o in range(2):
                nc.vector.reduce_sum(stp1[:, o:o + 1], st1[:, o, :], axis=AX.X)
                nc.vector.reduce_sum(
                    stp1[:, 2 + o:3 + o], st1sq[:, o, :], axis=AX.X
                )
            nc.sync.dma_start(cc1_in[:, :], stp1[:])
            nc.gpsimd.collective_compute(
                "AllGather", ALU.bypass, replica_groups=ALL8,
                ins=[cc1_in.opt()], outs=[cc1_out.opt()],
            )
            warm2 = psum.tile([P, 2, 512], F32, tag="q23", name="warm2")
            for _ in range(60):
                nc.tensor.matmul(warm2[:, 0, 0:C], Wp_sb[:, 0, 0:P], Wp_sb[:, 0, :],
                                 start=True, stop=True)
            warm2_scr = ppool.tile([P, 1], F32, tag="warm_scr", name="w2scr")
            nc.vector.reduce_sum(warm2_scr[:], warm2[:, 0, 0:4], axis=AX.X)
            stg1g = ppool.tile([P, NC, 4], F32, tag="stg1g")
            nc.sync.dma_start(
                stg1g[:],
                bass.AP(tensor=cc1_out.tensor, offset=cc1_out.offset,
                        ap=[[4, P], [P * 4, NC], [1, 4]]),
            )
            stg1 = ppool.tile([P, 4], F32, tag="stg1")
            for c4 in range(4):
                nc.vector.reduce_sum(
                    stg1[:, c4:c4 + 1], stg1g[:, :, c4], axis=AX.X
                )

            def bn_affine(stg, gam, bet, tagp, cnt, fold=1.0):
                """A, C with y = relu(x*A + C) == relu(fold*bn(x))."""
                mean = ppool.tile([P, 2], F32, tag=f"{tagp}_mean")
                nc.vector.tensor_scalar_mul(mean[:], stg[:, 0:2], 1.0 / cnt)
                ex2 = ppool.tile([P, 2], F32, tag=f"{tagp}_ex2")
                nc.vector.tensor_scalar_mul(ex2[:], stg[:, 2:4], 1.0 / cnt)
                var = ppool.tile([P, 2], F32, tag=f"{tagp}_var")
                nc.vector.tensor_tensor(var[:], mean[:], mean[:], ALU.mult)
                nc.vector.tensor_tensor(var[:], ex2[:], var[:], ALU.subtract)
                rstd = ppool.tile([P, 2], F32, tag=f"{tagp}_rstd")
                nc.vector.tensor_scalar_add(var[:], var[:], EPS)
                nc.scalar.activation(rstd[:], var[:], AF.Ln)
                nc.scalar.activation(rstd[:], rstd[:], AF.Exp, scale=-0.5)
                A = ppool.tile([P, 2], F32, tag=f"{tagp}_A")
                nc.vector.tensor_tensor(A[:], gam[:], rstd[:], ALU.mult)
                Cc = ppool.tile([P, 2], F32, tag=f"{tagp}_C")
                nc.vector.tensor_tensor(Cc[:], mean[:], A[:], ALU.mult)
                nc.vector.tensor_tensor(Cc[:], bet[:], Cc[:], ALU.subtract)
                if fold != 1.0:
                    nc.vector.tensor_scalar_mul(A[:], A[:], fold)
                    nc.vector.tensor_scalar_mul(Cc[:], Cc[:], fold)
                return A, Cc

            A1, C1 = bn_affine(stg1, g1_sb, b1_sb, "bn1", CNT1)
            hbn = ppool.tile([P, 2, N], F16, tag="hbn")
            for w4 in range(4):
                for o in range(2):
                    sl = slice(w4 * 512, (w4 + 1) * 512)
                    nc.scalar.activation(
                        out=hbn[:, o, sl], in_=hT[:, o, sl], func=AF.Relu,
                        scale=A1[:, o:o + 1], bias=C1[:, o:o + 1],
                    )

            # ---------- phase E: hp projections ----------
            # hpTf: [d-major (eo*64+d), hh, n]  (fp16, e-matmul operands)
            hpTf = ppool.tile([P, 2, N], F16, tag="hpTf")
            for hh in range(2):
                for w4 in range(4):
                    psb = psum.tile([P, 2, 512], F32,
                                    tag=("q01", "q23")[(hh * 4 + w4) % 2],
                                    name="ps_hpf")
                    ps = psb[:, 0, :]
                    for o in range(2):
                        nc.tensor.matmul(
                            ps,
                            Wh_sb[:, o, hh, :],
                            hbn[:, o, w4 * 512:(w4 + 1) * 512],
                            start=(o == 0), stop=(o == 1),
                        )
                    nc.vector.tensor_copy(
                        out=hpTf[:, hh, w4 * 512:(w4 + 1) * 512], in_=ps
                    )
            # hpA: node-major [n-part, t, hh, 130] f32r with ones cols 64/129
            hpA = ppool.tile([P, 16, 2, 130], F32R, tag="hpA")
            ones1 = ppool.tile([P, 1], F32, tag="ones1")
            nc.vector.memset(ones1[:], 1.0)
            ones_src = bass.AP(
                tensor=ones1.tensor, offset=ones1.offset,
                ap=[ones1.ap[0], [0, 16], [0, 2]],
            )
            for col in (64, 129):
                onesv = bass.AP(
                    tensor=hpA.tensor, offset=hpA.offset + col,
                    ap=[hpA.ap[0], [260, 16], [130, 2]],
                )  # [p, t, hh] at fixed col
                nc.vector.tensor_copy(out=onesv, in_=ones_src)
            for t in range(16):
                psb = psum.tile([P, 2, 512], F32, tag=("agg01", "x01")[t % 2],
                                name="ps_hpa")
                ps = psb[:, 0, 0:C]
                for o in range(2):
                    nc.tensor.matmul(
                        ps,
                        hbn[:, o, t * P:(t + 1) * P],
                        Wh_sb[:, o, :, :].rearrange("p hh x -> p (hh x)"),
                        start=(o == 0), stop=(o == 1),
                    )
                # psum cols (hh, eo, d) -> hpA[:, t, hh, eo*65 + d]
                psv = ps.rearrange("p (hh eo d) -> p hh eo d", hh=2, eo=2)
                dst = bass.AP(
                    tensor=hpA.tensor, offset=hpA.offset + t * 260,
                    ap=[hpA.ap[0], [130, 2], [65, 2], [1, 64]],
                )
                if t % 2 == 0:
                    nc.scalar.activation(out=dst, in_=psv[:], func=AF.Copy)
                else:
                    nc.vector.tensor_copy(out=dst, in_=psv[:])

            # ---------- phase F: attention fused with g/conv/bn2-stats ------
            aggT = ppool.tile([P, 2, Q], F32, tag="aggT")
            den4 = ppool.tile([4, Q], F32, tag="den4")
            rec4f = ppool.tile([4, Q], F32, tag="rec4f")
            rscr = ppool.tile([4, Q], F32, tag="rscr")
            rec4 = ppool.tile([4, Q], F32R, tag="rec4")
            g = ppool.tile([P, 2, Q], F16, tag="g")
            tconv = ppool.tile([P, 2, R], F16, tag="tconv")
            st2 = ppool.tile([P, 2, 3], F32, tag="st2")
            st2sq = ppool.tile([P, 2, 3], F32, tag="st2sq")
            emv = bass.AP(
                tensor=em_sb.tensor, offset=em_sb.offset,
                ap=[em_sb.ap[0], [0, 2], [1, 1]],
            )  # [p, o(bc), 1] at left-mask col; offset +1 for right

            def attn_block(hh, qb):
                """e/lrelu/exp/agg for head pair hh, query cols [qb, qb+512)."""
                agg2 = psum.tile([P, 2, 512], F32, tag="agg01", name="agg2")
                for g4 in range(4):
                    # el is eo-major so the in-place exp runs on contiguous
                    # [P, 4, 512] slabs (the hardware-validated pattern)
                    el = expool.tile([P, 2, 4, 512], F32R, tag="el")
                    for tt in range(4):
                        t = g4 * 4 + tt
                        eg2 = psum.tile([P, 2, 512], F32,
                                        tag=("q01", "q23")[t % 2], name="eg2")
                        nc.tensor.matmul(
                            eg2[:, 0, :],
                            hpTf[0:64, hh, t * P:(t + 1) * P],
                            hpTf[0:64, hh, qb:qb + 512],
                            start=True, stop=True,
                        )
                        nc.tensor.matmul(
                            eg2[:, 1, :],
                            hpTf[64:128, hh, t * P:(t + 1) * P],
                            hpTf[64:128, hh, qb:qb + 512],
                            start=True, stop=True,
                        )
                        # leaky-relu evacuation psum -> sbuf f32r; some tiles
                        # take the ACT parametric-relu path to balance engines
                        if t in ACT_T:
                            for eo in range(2):
                                nc.scalar.activation(
                                    out=el[:, eo, tt, :], in_=eg2[:, eo, :],
                                    func=AF.Prelu, alpha=SLOPE,
                                )
                        else:
                            nc.vector._custom_dve(
                                LRELU_ANT, out=el[:, :, tt, :], in0=eg2[:],
                                s0=SLOPE,
                            )
                    for eo in range(2):
                        nc.scalar.activation(out=el[:, eo, :, :],
                                             in_=el[:, eo, :, :],
                                             func=AF.Exp, bias=cm40[:])
                        for tt in range(4):
                            t = g4 * 4 + tt
                            nc.tensor.matmul(
                                agg2[0:65, eo, :],
                                hpA[:, t, hh, eo * 65:(eo + 1) * 65],
                                el[:, eo, tt, :],
                                start=(t == 0), stop=(t == 15),
                            )
                # drain agg (psum -> sbuf stage -> aggT/den rows via DMA)
                stage2 = wpool.tile([65, 2, 512], F32, tag="stage",
                                    name="stage2")
                nc.vector.tensor_copy(out=stage2[:], in_=agg2[0:65, :, :])
                nc.sync.dma_start(aggT[0:64, hh, qb:qb + 512], stage2[0:64, 0, :])
                nc.sync.dma_start(
                    den4[2 * hh:2 * hh + 1, qb:qb + 512], stage2[64:65, 0, :]
                )
                nc.sync.dma_start(aggT[64:128, hh, qb:qb + 512], stage2[0:64, 1, :])
                nc.sync.dma_start(
                    den4[2 * hh + 1:2 * hh + 2, qb:qb + 512], stage2[64:65, 1, :]
                )

            def g_chunk(qb, ws):
                """recip + g assembly for query cols [qb, qb+ws)."""
                nc.vector.reciprocal_approx_accurate(
                    out=rec4f[:, qb:qb + ws], in_=den4[:, qb:qb + ws],
                    scratch=rscr[:, qb:qb + ws],
                )
                nc.vector.tensor_scalar_mul(
                    rec4[:, qb:qb + ws], rec4f[:, qb:qb + ws], float(alpha_gat)
                )
                for o in range(2):
                    bcb = psum.tile([P, 2, 512], F32, tag="x01", name="bc")
                    bc = bcb[:, 0, 0:ws]
                    nc.tensor.matmul(
                        bc,
                        Ebc_sb[:, o * P:(o + 1) * P],
                        rec4[:, qb:qb + ws],
                        start=True, stop=True,
                    )
                    gsl = g[:, o, qb:qb + ws]
                    nc.vector.tensor_tensor(
                        gsl, aggT[:, o, qb:qb + ws], bc, ALU.mult
                    )
                    nc.vector._custom_dve(
                        dve_ops.AFFINE_THEN_ADD,
                        out=gsl,
                        in0=hbn[:, o, qb:qb + ws],
                        in1=gsl,
                        s0=float(1.0 - alpha_gat),
                        s1=0.0,
                    )

            def mask_edge(col, mi):
                """halo-edge validity mask on g col (mi: 0=left, 1=right)."""
                gsl = bass.AP(
                    tensor=g.tensor, offset=g.offset + col,
                    ap=[g.ap[0], [Q, 2], [1, 1]],
                )
                mv = bass.AP(
                    tensor=em_sb.tensor, offset=em_sb.offset + mi,
                    ap=[em_sb.ap[0], [0, 2], [1, 1]],
                )
                nc.vector.tensor_tensor(gsl, gsl, mv, ALU.mult)

            CONV_CHUNKS = ((1, 511), (511, 1021), (1021, 1025))

            def conv_chunk(ci):
                """conv output cols [a, b) -> tconv cols [a-1, b-1) + stats."""
                a, b_ = CONV_CHUNKS[ci]
                width = b_ - a
                cvb = psum.tile([P, 2, 512], F32, tag="x01", name="ps_cv")
                for oo in range(2):
                    first = True
                    for oi in range(2):
                        for k in range(3):
                            nc.tensor.matmul(
                                cvb[:, oo, 0:width],
                                Wk_sb[:, oi, k, oo * P:(oo + 1) * P],
                                g[:, oi, a - 1 + k:b_ - 1 + k],
                                start=first, stop=(oi == 1 and k == 2),
                            )
                            first = False
                    # conv_b omitted: bn2 (training mode) follows directly, so
                    # a per-channel constant shift cancels exactly.
                    tsl = tconv[:, oo, a - 1:b_ - 1]
                    nc.scalar.activation(
                        out=tsl, in_=cvb[:, oo, 0:width], func=AF.Copy,
                        accum_out=st2[:, oo, ci:ci + 1],
                    )
                    nc.vector.affine_mul_reduce(
                        out=sqscr[:, 0:width],
                        accum_out=st2sq[:, oo, ci:ci + 1],
                        in0=tsl, in1=tsl, scale=1.0, bias=0.0,
                    )

            for hh in range(2):
                attn_block(hh, 0)
            g_chunk(0, 512)
            mask_edge(0, 0)
            conv_chunk(0)

            # ----- halo queries (cols 1024, 1025) -----
            # hoisted between the two main query chunks: they only need the
            # projections, and finishing them early shortens the serial tail
            # eh_ps/elH cols: hh*64 + eo*32 + t*2 + q
            ehb = psum.tile([P, 2, 512], F32, tag="q01", name="eh_ps")
            for hh in range(2):
                for eo in range(2):
                    sl = slice(eo * 64, (eo + 1) * 64)
                    for t in range(16):
                        cbase = hh * 64 + eo * 32 + t * 2
                        nc.tensor.matmul(
                            ehb[:, 0, cbase:cbase + 2],
                            hpTf[sl, hh, t * P:(t + 1) * P],
                            hpTf[sl, hh, R:R + 2],
                            start=True, stop=True,
                        )
            elH = expool.tile([P, 128], F32R, tag="elH")
            nc.vector._custom_dve(LRELU_ANT, out=elH[:], in0=ehb[:, 0, 0:128], s0=SLOPE)
            nc.scalar.activation(out=elH[:], in_=elH[:], func=AF.Exp,
                                 bias=cm40[:])
            aggHb = psum.tile([P, 2, 512], F32, tag="agg01", name="aggH")
            for hh in range(2):
                for eo in range(2):
                    for t in range(16):
                        cbase = hh * 64 + eo * 32 + t * 2
                        hbase = (2 * hh + eo) * 2
                        nc.tensor.matmul(
                            aggHb[0:65, 0, hbase:hbase + 2],
                            hpA[:, t, hh, eo * 65:(eo + 1) * 65],
                            elH[:, cbase:cbase + 2],
                            start=(t == 0), stop=(t == 15),
                        )
            stageH = wpool.tile([65, 8], F32, tag="stageH")
            nc.vector.tensor_copy(out=stageH[:], in_=aggHb[0:65, 0, 0:8])
            nc.sync.dma_start(den4[0:4, R:R + 2], stageH[64:65, :])
            for hh in range(2):
                nc.vector.tensor_copy(
                    out=aggT[0:64, hh, R:R + 2],
                    in_=stageH[0:64, 4 * hh:4 * hh + 2],
                )
                nc.sync.dma_start(
                    aggT[64:128, hh, R:R + 2],
                    stageH[0:64, 4 * hh + 2:4 * hh + 4],
                )

            # halo g column ready long before the tail
            g_chunk(R, 2)
            mask_edge(Q - 1, 1)

            # ----- second main query chunk + last conv chunks -----
            for hh in range(2):
                attn_block(hh, 512)
            g_chunk(512, 512)
            conv_chunk(1)
            conv_chunk(2)

            # ---------- phase I: BN2 + residual + output ----------
            stp2 = ppool.tile([P, 4], F32, tag="stp2")
            for o in range(2):
                nc.vector.reduce_sum(stp2[:, o:o + 1], st2[:, o, :], axis=AX.X)
                nc.vector.reduce_sum(
                    stp2[:, 2 + o:3 + o], st2sq[:, o, :], axis=AX.X
                )
            nc.sync.dma_start(cc2_in[:, :], stp2[:])
            nc.gpsimd.collective_compute(
                "AllGather", ALU.bypass, replica_groups=ALL8,
                ins=[cc2_in.opt()], outs=[cc2_out.opt()],
            )
            stg2g = ppool.tile([P, NC, 4], F32, tag="stg2g")
            nc.sync.dma_start(
                stg2g[:],
                bass.AP(tensor=cc2_out.tensor, offset=cc2_out.offset,
                        ap=[[4, P], [P * 4, NC], [1, 4]]),
            )
            stg2 = ppool.tile([P, 4], F32, tag="stg2")
            for c4 in range(4):
                nc.vector.reduce_sum(
                    stg2[:, c4:c4 + 1], stg2g[:, :, c4], axis=AX.X
                )
            fold = alpha_tcn if alpha_tcn > 0 else 1.0
            A2, C2 = bn_affine(stg2, g2_sb, b2_sb, "bn2", CNT2, fold=fold)

            fin = ppool.tile([P, 2, R], F32, tag="fin")
            HR = R // 2
            for o in range(2):
                for q2 in range(2):
                    sl = slice(q2 * HR, (q2 + 1) * HR)
                    nc.scalar.activation(
                        out=fin[:, o, sl], in_=tconv[:, o, sl], func=AF.Relu,
                        scale=A2[:, o:o + 1], bias=C2[:, o:o + 1],
                    )
                    if fold != alpha_tcn:  # alpha_tcn <= 0: scale separately
                        nc.vector.tensor_scalar_mul(
                            fin[:, o, sl], fin[:, o, sl], float(alpha_tcn)
                        )
                    nc.vector._custom_dve(
                        dve_ops.AFFINE_THEN_ADD,
                        out=fin[:, o, sl],
                        in0=g[:, o, 1 + q2 * HR:1 + (q2 + 1) * HR],
                        in1=fin[:, o, sl],
                        s0=float(1.0 - alpha_tcn),
                        s1=0.0,
                    )
                    nc.sync.dma_start(out[:, o, sl], fin[:, o, sl])

    nc.compile()
    return nc


def _prep_inputs(x, adj, W_sage, b_sage, bn1_gamma, bn1_beta, Wh,
                 conv_w, bn2_gamma, bn2_beta, conv_b=None):
    """Build the 8 per-core input maps (host-side numpy)."""
    x = np.asarray(x, np.float32)
    adj = np.asarray(adj, np.float32)
    Wh = np.asarray(Wh, np.float32)          # [H, C, DH]
    WhT = np.zeros((C, 2 * P), np.float32)   # cols hh*128 + eo*64 + d
    for hh in range(2):
        for eo in range(2):
            WhT[:, hh * P + eo * DH:hh * P + eo * DH + DH] = Wh[2 * hh + eo]
    WkT = np.ascontiguousarray(np.asarray(conv_w, np.float32).transpose(2, 1, 0))
    Ebc = np.zeros((4, C), np.float32)
    for c in range(C):
        Ebc[(c % P) // DH + 2 * (c // P), c] = 1.0
    if conv_b is None:
        cbv = np.zeros((C,), np.float32)
    else:
        cbv = np.asarray(conv_b, np.float32)

    shared = dict(
        Wp=np.asarray(W_sage, np.float16),
        bs=np.asarray(b_sage, np.float32),
        g1=np.asarray(bn1_gamma, np.float32),
        b1=np.asarray(bn1_beta, np.float32),
        WhT=WhT.astype(np.float16),
        WkT=WkT.astype(np.float16),
        cb=cbv,
        g2=np.asarray(bn2_gamma, np.float32),
        b2=np.asarray(bn2_beta, np.float32),
        Ebc=Ebc,
    )
    adjT = adj.T
    in_maps = []
    for core in range(NC):
        b, s = core // 2, core % 2
        shift = s * R - 1  # rotated col i -> global node (i + shift) mod N
        xTr = np.roll(x[b].T, -shift, axis=1)
        adjTr = np.roll(np.roll(adjT, -shift, axis=0), -shift, axis=1)
        emask_v = np.array(
            [0.0 if s == 0 else 1.0, 1.0 if s == 0 else 0.0], np.float32
        )
        m = dict(
            xT=xTr.astype(np.float16),
            adjT=adjTr.astype(np.float16),
            emask=emask_v,
            **shared,
        )
        in_maps.append(m)
    return in_maps


def _assemble(results):
    out = np.empty((B, N, C), np.float32)
    for core in range(NC):
        b, s = core // 2, core % 2
        r = results[core]["out"]  # [P, 2, R]
        out[b, s * R:(s + 1) * R, :] = r.transpose(2, 1, 0).reshape(R, C)
    return out


_CACHE = {}


def kernel(x, adj, W_sage, b_sage, bn1_gamma, bn1_beta, Wh, alpha_gat,
           conv_w, conv_b, bn2_gamma, bn2_beta, alpha_tcn, **_unused):
    ag, at = float(alpha_gat), float(alpha_tcn)
    key = (ag, at)
    if key not in _CACHE:
        _CACHE[key] = build_program(ag, at)
    nc = _CACHE[key]
    in_maps = _prep_inputs(x, adj, W_sage, b_sage, bn1_gamma, bn1_beta, Wh,
                           conv_w, bn2_gamma, bn2_beta, conv_b)
    res = run_bass_kernel_spmd(nc, in_maps, core_ids=list(range(NC)))
    return _assemble(res.results)


if __name__ == "__main__":
    import sys
    sys.path.insert(0, "/root/problem")
    import reference
    inputs = {k: np.asarray(v) for k, v in reference.setup_inputs().items()}
    expected = np.asarray(reference.reference(**inputs))
    actual = kernel(**inputs)
    err = np.abs(actual - expected)
    rel = np.linalg.norm(actual - expected) / np.linalg.norm(expected)
    print("max abs err:", err.max(), "rel:", rel)
